# revision 1
# baseline (speedup 1.0000x reference)
"""DiffPoolEncoder Trainium2 kernel.

Sharding: data parallel by graph. 8 cores x 4 graphs (512 nodes each).
Per core the whole network runs on-device; GraphSage aggregation uses dense
per-graph A^T tiles built on-device via gpsimd local_scatter from
host-prepared (dst, count) index tables (index-only preprocessing of the
edge list). Heavy matmuls run in float32r (TF32, 1 cycle/row); adjacency
counts and 1/deg (deg = 2^k) are exactly representable, so the aggregation
matrix itself is exact. Activations keep dual layout: feature-major from
the linears, node-major via PE transposes. SBUF pools are strictly
LIFO-nested by lifetime epoch.
"""

import sys

for _p in ("/opt/trn_rl_repo",):
    if _p not in sys.path:
        sys.path.append(_p)

import numpy as np
import ml_dtypes
from contextlib import ExitStack

import concourse.bass as bass
import concourse.mybir as mybir
import concourse.tile as tile
from concourse import bacc
from concourse.bass_utils import run_bass_kernel_spmd

F32 = mybir.dt.float32
F32R = mybir.dt.float32r
BF16 = mybir.dt.bfloat16
I16 = mybir.dt.int16
AF = mybir.ActivationFunctionType
ALU = mybir.AluOpType
AX = mybir.AxisListType

NCORES = 8
B = 32
NPG = 512
G = 4            # graphs per core
T = 16           # node tiles per core (4 per graph)
NLOC = 2048      # nodes per core
K = 64           # clusters per graph
IN = 128
HID = 256
NI = 48          # padded (dst,count) entries per (src-tile, partition)

# bcol column layout (each 128-chunk of a bias vector is one column)
BC_B1, BC_B2, BC_B3 = 0, 2, 4
BC_AB1, BC_AB2 = 6, 8
BC_AB3 = 10          # 16 cols
BC_QB1, BC_QB2, BC_QB3 = 26, 28, 30
BC_MB1, BC_MB2 = 32, 34
BC_N = 35

# rows2 [65, 1536] f32r: rows at matmul base partitions {0, 32, 64};
# ones[0:512] replicated at each used partition (matmul needs equal bases).
R_QB1 = (0, 512)
R_QB2, R_QB3 = (64, 512), (64, 768)
R_PB = (32, 512)     # 256 (per-core pW bias slice)
ROWS_W = 1024


def build_module():
    nc = bacc.Bacc("TRN2", target_bir_lowering=False)

    # ---------------- DRAM I/O ----------------
    featT_d = nc.dram_tensor("featT", [128, NLOC], F32R, kind="ExternalInput")
    featnm_d = nc.dram_tensor("feat_nm", [128, T * IN], F32R, kind="ExternalInput")
    atidx_d = nc.dram_tensor("at_idx", [128, T * NI], I16, kind="ExternalInput")
    atval_d = nc.dram_tensor("at_val", [128, T * NI], BF16, kind="ExternalInput")
    bcol_d = nc.dram_tensor("bcol", [128, BC_N], F32, kind="ExternalInput")
    rows_d = nc.dram_tensor("rows2", [65, ROWS_W], F32R, kind="ExternalInput")
    ident_d = nc.dram_tensor("ident", [128, 128], F32, kind="ExternalInput")
    identr_d = nc.dram_tensor("identr", [128, 128], F32R, kind="ExternalInput")
    w_d = {}
    for name, fi, fo in [
        ("W1", 256, 256), ("W2", 512, 256), ("W3", 512, 256),
        ("aW1", 256, 256), ("aW2", 512, 256), ("aW3", 512, 2048),
        ("pWl", 2560, 256), ("qW1", 1536, 256), ("qW2", 512, 256),
        ("qW3", 512, 256), ("mW1", 1536, 256), ("mW2", 256, 10),
    ]:
        w_d[name] = nc.dram_tensor(name, [fi, fo], F32R, kind="ExternalInput")
    yp_d = nc.dram_tensor("yp", [10, G], F32, kind="ExternalOutput")

    with tile.TileContext(nc) as tc, ExitStack() as ex, \
            nc.allow_low_precision(reason="f32r is tf32; accumulation stays fp32 in PSUM"):
        persist = ex.enter_context(tc.tile_pool(name="persist", bufs=1))
        # PSUM: 8 banks. One tag per pool so slot count == bank count.
        ps_p = ex.enter_context(tc.tile_pool(name="psP", bufs=4, space="PSUM"))
        lg_p = ex.enter_context(tc.tile_pool(name="psL", bufs=1, space="PSUM"))
        pm_p = ex.enter_context(tc.tile_pool(name="psM", bufs=2, space="PSUM"))
        pl_p = ex.enter_context(tc.tile_pool(name="psS", bufs=1, space="PSUM"))
        dram = ex.enter_context(tc.tile_pool(name="dram", bufs=1, space="DRAM"))

        uid = [0]

        def _nm(pfx):
            uid[0] += 1
            return f"{pfx}{uid[0]}"

        def ps_big(dt=F32):
            return ps_p.tile([128, 512], dt, tag="ps", name=_nm("ps"))

        def ps_med(p, f, dt=F32):
            return pm_p.tile([p, f], dt, tag="pm", name=_nm("pm"))

        def ps_sml(p, f, dt=F32):
            return pl_p.tile([p, f], dt, tag="pl", name=_nm("pl"))

        def wload(pool, name, fi, fo):
            kk = fi // 128
            sb = pool.tile([128, kk * fo], F32R, tag=name, name=name)
            nc.sync.dma_start(
                sb[:].rearrange("p (k f) -> p k f", k=kk, f=fo),
                w_d[name][:, :].rearrange("(k p) f -> p k f", p=128),
            )
            return sb

        # ---------- persistent small tensors (epoch E6) ----------
        ident = persist.tile([128, 128], F32)
        identr = persist.tile([128, 128], F32R)
        rows2 = persist.tile([65, ROWS_W], F32R)
        bcol = persist.tile([128, BC_N], F32)
        ones_c = persist.tile([128, 1], F32R)
        degcl = persist.tile([128, T], F32)      # clamped deg, node-major cols
        S_nm = persist.tile([128, T * K], F32R)
        out_fm = persist.tile([128, 12 * G], F32R)  # readout maxes, col=ch*G+g
        nmax = persist.tile([128, 2], F32)
        sumx = persist.tile([128, 2], F32)
        y_sb = persist.tile([128, 2 * G], F32R)
        z_sb = persist.tile([10, G], F32)
        nc.sync.dma_start(ident[:], ident_d[:])
        nc.sync.dma_start(identr[:], identr_d[:])
        nc.sync.dma_start(rows2[:], rows_d[:])
        nc.sync.dma_start(bcol[:], bcol_d[:])
        nc.vector.memset(ones_c[:].bitcast(F32), 1.0)

        def ones_at(p, n):
            return rows2[p : p + 1, 0:n]

        def rrow(ro, n):
            p, off = ro
            return rows2[p : p + 1, off : off + n]

        # AT (scaled A^T tiles, f32) lives to the end (epoch E6)
        at_p = ex.enter_context(tc.tile_pool(name="atp", bufs=1))
        AT = at_p.tile([128, T * NPG], F32R)

        # ---------- LIFO phase pools ----------
        ex5 = ExitStack()   # close after logits/softmax
        agg_p = ex5.enter_context(tc.tile_pool(name="aggfm", bufs=2))
        afm2_p = ex5.enter_context(tc.tile_pool(name="afm2", bufs=1))
        ex3 = ExitStack()   # close after h3 (h3 runs after xnm closes)
        xfm_p = ex3.enter_context(tc.tile_pool(name="xfm", bufs=2))
        w3_p = ex3.enter_context(tc.tile_pool(name="w3p", bufs=1))
        ex4 = ExitStack()   # close after agg_a2
        xnm_p = ex4.enter_context(tc.tile_pool(name="xnm", bufs=2))
        ex2c = ExitStack()  # close after a2
        a2w_p = ex2c.enter_context(tc.tile_pool(name="a2wp", bufs=1))
        afm1_p = ex2c.enter_context(tc.tile_pool(name="afm1", bufs=1))
        ex2b = ExitStack()  # close after h2
        w2_p = ex2b.enter_context(tc.tile_pool(name="w2p", bufs=1))
        ex2 = ExitStack()   # close after h1/a1
        w1_p = ex2.enter_context(tc.tile_pool(name="w1p", bufs=1))
        ex1 = ExitStack()   # close after agg_feat
        fnm_p = ex1.enter_context(tc.tile_pool(name="fnmp", bufs=1))
        ex0 = ExitStack()   # close after A^T built+scaled
        ate_p = ex0.enter_context(tc.tile_pool(name="atep", bufs=1))

        # ---------- input DMAs ----------
        atbf_idx = ate_p.tile([128, T * NI], I16, tag="atidx")
        atbf_val = ate_p.tile([128, T * NI], BF16, tag="atval")
        nc.sync.dma_start(atbf_idx[:], atidx_d[:])
        nc.sync.dma_start(atbf_val[:], atval_d[:])
        featnm = fnm_p.tile([128, T * IN], F32R, tag="featnm")
        nc.sync.dma_start(featnm[:], featnm_d[:])
        W1 = wload(w1_p, "W1", 256, 256)
        aW1 = wload(w1_p, "aW1", 256, 256)
        W2 = wload(w2_p, "W2", 512, 256)
        W3 = wload(w3_p, "W3", 512, 256)
        aW2 = wload(a2w_p, "aW2", 512, 256)

        # ---------- phase 0: build scaled A^T ----------
        for t in range(T):
            scr = ate_p.tile([128, NPG], BF16, tag="scscr", name=_nm("sc"), bufs=1)
            nc.gpsimd.local_scatter(
                out_ap=scr[:],
                data_ap=atbf_val[:, t * NI : (t + 1) * NI],
                idxs_ap=atbf_idx[:, t * NI : (t + 1) * NI],
                channels=128, num_elems=NPG, num_idxs=NI,
            )
            nc.vector.tensor_copy(AT[:, t * NPG : (t + 1) * NPG], scr[:])

        # deg rows -> dinv rows (partitions 0/32 of a [33,1024] tile)
        dinvsb = ate_p.tile([33, 1024], F32R, tag="dinvsb")
        gslot = [(0, 0), (0, 512), (32, 0), (32, 512)]
        for g in range(G):
            p, off = gslot[g]
            dps = ps_big()
            for st in range(4):
                nc.tensor.matmul(dps[p : p + 1, :], lhsT=ones_c[:].bitcast(F32),
                                 rhs=AT[:, (g * 4 + st) * NPG : (g * 4 + st + 1) * NPG].bitcast(F32),
                                 start=(st == 0), stop=(st == 3),
                                 skip_group_check=True)
            nc.vector.tensor_scalar(dinvsb[p : p + 1, off : off + NPG],
                                    dps[p : p + 1, :], 1.0, None, op0=ALU.max)
            nc.vector.reciprocal(dinvsb[p : p + 1, off : off + NPG],
                                 dinvsb[p : p + 1, off : off + NPG])
        # clamped deg as node-major columns (from raw A^T)
        for t in range(T):
            g, j = t // 4, t % 4
            cps = ps_sml(128, 1)
            for st in range(4):
                nc.tensor.matmul(
                    cps[:],
                    lhsT=AT[:, (g * 4 + st) * NPG + j * 128 : (g * 4 + st) * NPG + (j + 1) * 128].bitcast(F32),
                    rhs=ones_c[:].bitcast(F32), start=(st == 0), stop=(st == 3))
            nc.vector.tensor_scalar(degcl[:, t : t + 1], cps[:], 1.0, None, op0=ALU.max)
        # AT <- AT * dinv[dst] (broadcast via K=1 outer product, per graph)
        for g in range(G):
            p, off = gslot[g]
            bps = ps_big()
            nc.tensor.matmul(bps[:], lhsT=ones_at(p, 128).bitcast(F32),
                             rhs=dinvsb[p : p + 1, off : off + NPG].bitcast(F32),
                             start=True, stop=True)
            dbc = ate_p.tile([128, NPG], F32R, tag="dbc", name=_nm("dbc"), bufs=1)
            nc.scalar.copy(dbc[:], bps[:])
            for st in range(4):
                t = g * 4 + st
                nc.vector.tensor_tensor(
                    out=AT[:, t * NPG : (t + 1) * NPG],
                    in0=AT[:, t * NPG : (t + 1) * NPG],
                    in1=dbc[:], op=ALU.mult)
        ex0.close()
        ft_p_stack = ExitStack()   # close after h1/a1
        ft_p = ft_p_stack.enter_context(tc.tile_pool(name="ftp", bufs=1))
        featT = ft_p.tile([128, NLOC], F32R, tag="featT")
        nc.sync.dma_start(featT[:], featT_d[:])

        # ---------- emit helpers ----------
        def emit_agg(x_nm, D, out_t):
            """out_t[d, n] (feature-major) = sum_s x_nm[s, d] * AT[s, n]."""
            for g in range(G):
                for ch in range(D // 128):
                    ps = ps_big()
                    for st in range(4):
                        t = g * 4 + st
                        nc.tensor.matmul(
                            ps[:],
                            lhsT=x_nm[:, t * D + ch * 128 : t * D + ch * 128 + 128],
                            rhs=AT[:, t * NPG : (t + 1) * NPG],
                            start=(st == 0), stop=(st == 3))
                    nc.scalar.copy(
                        out_t[:, ch * NLOC + g * NPG : ch * NLOC + (g + 1) * NPG],
                        ps[:])

        def emit_lin_fm(x_fm, a_fm, Din, Dout, Wsb, bccol, relu, out_t):
            nk = Din // 128
            for co in range(Dout // 128):
                for nb in range(4):
                    ps = ps_big()
                    ki = 0
                    for src in (x_fm, a_fm):
                        for ci in range(nk):
                            nc.tensor.matmul(
                                ps[:],
                                lhsT=Wsb[:, ki * Dout + co * 128 : ki * Dout + co * 128 + 128],
                                rhs=src[:, ci * NLOC + nb * 512 : ci * NLOC + (nb + 1) * 512],
                                start=(ki == 0), stop=(ki == 2 * nk - 1))
                            ki += 1
                    nc.scalar.activation(
                        out_t[:, co * NLOC + nb * 512 : co * NLOC + (nb + 1) * 512],
                        ps[:], AF.Relu if relu else AF.Identity,
                        bias=bcol[:, bccol + co : bccol + co + 1])

        def emit_nm_T(x_fm, out_nm, act=False, spill=None):
            # node-major via PE transposes of the (already relu'd) fm tensor;
            # 4 transposed blocks share one PSUM bank -> single 512-wide evac.
            for t2 in range(0, T, 2):
                tp = ps_big(F32R)
                for i, (t, ch) in enumerate(
                        ((t2, 0), (t2, 1), (t2 + 1, 0), (t2 + 1, 1))):
                    nc.tensor.matmul(
                        tp[:, i * 128 : (i + 1) * 128],
                        lhsT=x_fm[:, ch * NLOC + t * 128 : ch * NLOC + (t + 1) * 128],
                        rhs=identr[:], is_transpose=True,
                        start=True, stop=True, skip_group_check=True)
                if spill is not None:
                    buf, dst_dram = spill
                    sb = buf()
                    nc.vector.tensor_copy(sb[:], tp[:])
                    nc.sync.dma_start(
                        dst_dram[:, t2 * HID : (t2 + 2) * HID], sb[:])
                else:
                    dst = out_nm[:, t2 * HID : (t2 + 2) * HID]
                    if act:
                        nc.scalar.copy(dst, tp[:])
                    else:
                        nc.vector.tensor_copy(dst, tp[:])

        def emit_out1(x_fm, ch0):
            for ci in range(2):
                for g in range(G):
                    nc.vector.tensor_reduce(
                        out_fm[:, (ch0 + ci) * G + g : (ch0 + ci) * G + g + 1],
                        x_fm[:, ci * NLOC + g * NPG : ci * NLOC + (g + 1) * NPG],
                        axis=AX.X, op=ALU.max)

        # ---------- GC stacks ----------
        aggfeat = agg_p.tile([128, NLOC], F32R, tag="agg", name="aggfeat")
        emit_agg(featnm, IN, aggfeat)

        h1f = xfm_p.tile([128, 2 * NLOC], F32R, tag="xfm", name="h1f")
        h1n = xnm_p.tile([128, T * HID], F32R, tag="xnm", name="h1n")
        emit_lin_fm(featT, aggfeat, 128, 256, W1, BC_B1, True, h1f)
        emit_nm_T(h1f, h1n)
        h1spill = dram.tile([128, T * HID], F32R, tag="h1d", name="h1d")
        nc.sync.dma_start(h1spill[:], h1n[:])
        emit_out1(h1f, 0)

        a1f = afm1_p.tile([128, 2 * NLOC], F32R, tag="a1f", name="a1f")
        a1n = xnm_p.tile([128, T * HID], F32R, tag="xnm", name="a1n")
        emit_lin_fm(featT, aggfeat, 128, 256, aW1, BC_AB1, True, a1f)
        a1spill = dram.tile([128, 2 * NLOC], F32R, tag="a1d", name="a1d")
        nc.sync.dma_start(a1spill[:], a1f[:])
        emit_nm_T(a1f, a1n, act=True)
        ft_p_stack.close()
        ex1.close()
        ex2.close()

        aggh1 = agg_p.tile([128, 2 * NLOC], F32R, tag="agg", name="aggh1")
        emit_agg(h1n, HID, aggh1)

        agga1 = agg_p.tile([128, 2 * NLOC], F32R, tag="agg", name="agga1")
        emit_agg(a1n, HID, agga1)

        h2f = xfm_p.tile([128, 2 * NLOC], F32R, tag="xfm", name="h2f")
        h2n = xnm_p.tile([128, T * HID], F32R, tag="xnm", name="h2n")
        emit_lin_fm(h1f, aggh1, 256, 256, W2, BC_B2, True, h2f)
        emit_nm_T(h2f, h2n)
        h2spill = dram.tile([128, T * HID], F32R, tag="h2d", name="h2d")
        nc.sync.dma_start(h2spill[:], h2n[:])
        emit_out1(h2f, 2)
        ex2b.close()

        a2f = afm2_p.tile([128, 2 * NLOC], F32R, tag="a2f", name="a2f")
        a2n = xnm_p.tile([128, T * HID], F32R, tag="xnm", name="a2n")
        emit_lin_fm(a1f, agga1, 256, 256, aW2, BC_AB2, True, a2f)
        emit_nm_T(a2f, a2n, act=True)
        ex2c.close()

        aggh2 = agg_p.tile([128, 2 * NLOC], F32R, tag="agg", name="aggh2")
        emit_agg(h2n, HID, aggh2)

        agga2 = agg_p.tile([128, 2 * NLOC], F32R, tag="agg", name="agga2")
        emit_agg(a2n, HID, agga2)
        ex4.close()

        # late weights issue here so their DMA overlaps h3 compute
        ex5b = ExitStack()
        wl_p = ex5b.enter_context(tc.tile_pool(name="wlate", bufs=1))
        aW3 = wl_p.tile([128, 4 * 2048], F32R, tag="aW3", name="aW3")
        aW3v = aW3[:].rearrange("p (k f) -> p k f", k=4, f=2048)
        for q in range(4):
            nc.sync.dma_start(
                aW3v[:, :, q * 512 : (q + 1) * 512],
                w_d["aW3"][:, q * 512 : (q + 1) * 512].rearrange(
                    "(k p) f -> p k f", p=128))
        pWl = wload(wl_p, "pWl", 2560, 256)
        a1r = wl_p.tile([128, 2 * NLOC], F32R, tag="a1r", name="a1r")
        nc.sync.dma_start(a1r[:], a1spill[:])
        lgs_nm = wl_p.tile([128, T * K], F32, tag="lgs", name="lgs_nm")

        # h3: fm + readout; node-major streamed straight to DRAM
        h3f = xfm_p.tile([128, 2 * NLOC], F32R, tag="xfm", name="h3f")
        h3spill = dram.tile([128, T * HID], F32R, tag="h3d", name="h3d")
        emit_lin_fm(h2f, aggh2, 256, 256, W3, BC_B3, False, h3f)

        def h3buf():
            return w3_p.tile([128, 512], F32R, tag="h3buf", name=_nm("h3b"), bufs=2)

        emit_nm_T(h3f, None, spill=(h3buf, h3spill))
        emit_out1(h3f, 4)

        # ---------- a3 + logits (streamed per graph) ----------
        for g in range(G):
            lps = lg_p.tile([64, 512], F32, tag="lg", name=_nm("lg"))
            for co in range(16):  # a3 = relu(cat(a2, agg_a2) @ aW3 + ab3)
                ps3 = ps_big()
                ki = 0
                for src in (a2f, agga2):
                    for ci in range(2):
                        nc.tensor.matmul(
                            ps3[:],
                            lhsT=aW3[:, ki * 2048 + co * 128 : ki * 2048 + co * 128 + 128],
                            rhs=src[:, ci * NLOC + g * NPG : ci * NLOC + (g + 1) * NPG],
                            start=(ki == 0), stop=(ki == 3))
                        ki += 1
                a3b = wl_p.tile([128, 512], F32R, tag="a3buf", name=_nm("a3b"), bufs=2)
                nc.scalar.activation(a3b[:], ps3[:], AF.Relu,
                                     bias=bcol[:, BC_AB3 + co : BC_AB3 + co + 1])
                nc.tensor.matmul(
                    lps[:], lhsT=pWl[:, (4 + co) * 256 + g * K : (4 + co) * 256 + g * K + K],
                    rhs=a3b[:], start=(co == 0), stop=False)
            for ci in range(2):  # a1 block of pW
                nc.tensor.matmul(
                    lps[:], lhsT=pWl[:, ci * 256 + g * K : ci * 256 + g * K + K],
                    rhs=a1r[:, ci * NLOC + g * NPG : ci * NLOC + (g + 1) * NPG],
                    start=False, stop=False)
            for ci in range(2):  # a2 block
                nc.tensor.matmul(
                    lps[:], lhsT=pWl[:, (2 + ci) * 256 + g * K : (2 + ci) * 256 + g * K + K],
                    rhs=a2f[:, ci * NLOC + g * NPG : ci * NLOC + (g + 1) * NPG],
                    start=False, stop=False)
            nc.tensor.matmul(lps[:],
                             lhsT=rows2[32:33, R_PB[1] + g * K : R_PB[1] + (g + 1) * K].bitcast(F32),
                             rhs=ones_at(32, 512).bitcast(F32), start=False, stop=True)
            lgf = wl_p.tile([64, 512], F32, tag="lgf", name=_nm("lgf"), bufs=1)
            nc.scalar.copy(lgf[:], lps[:])
            for j in range(4):  # transpose to node-major
                t = g * 4 + j
                tps = ps_med(128, 64)
                nc.tensor.transpose(tps[:], lgf[0:64, j * 128 : (j + 1) * 128],
                                    ident[0:64, 0:64])
                nc.vector.tensor_copy(lgs_nm[:, t * K : (t + 1) * K], tps[:])

        # masked softmax == per-graph softmax over K columns
        for t in range(T):
            bb = t % 2
            nc.vector.tensor_reduce(nmax[:, bb : bb + 1], lgs_nm[:, t * K : (t + 1) * K],
                                    axis=AX.X, op=ALU.max, negate=True)
            nc.scalar.activation(S_nm[:, t * K : (t + 1) * K],
                                 lgs_nm[:, t * K : (t + 1) * K], AF.Exp,
                                 bias=nmax[:, bb : bb + 1],
                                 accum_out=sumx[:, bb : bb + 1])
            nc.vector.reciprocal(sumx[:, bb : bb + 1], sumx[:, bb : bb + 1])
            nc.vector.tensor_scalar(S_nm[:, t * K : (t + 1) * K],
                                    S_nm[:, t * K : (t + 1) * K],
                                    sumx[:, bb : bb + 1], None, op0=ALU.mult)
        ex5b.close()
        ex3.close()
        ex5.close()

        # ---------- late pool: pooled stage ----------
        late = ex.enter_context(tc.tile_pool(name="late", bufs=1))
        h1r = late.tile([128, T * HID], F32R, tag="h1r", name="h1r")
        h2r = late.tile([128, T * HID], F32R, tag="h2r", name="h2r")
        h3r = late.tile([128, T * HID], F32R, tag="h3r", name="h3r")
        for q in range(4):
            sl = slice(q * 4 * HID, (q + 1) * 4 * HID)
            nc.sync.dma_start(h1r[:, sl], h1spill[:, sl])
            nc.sync.dma_start(h2r[:, sl], h2spill[:, sl])
            nc.sync.dma_start(h3r[:, sl], h3spill[:, sl])
        Xr = [h1r, h2r, h3r]
        qW1 = wload(late, "qW1", 1536, 256)
        qW2 = wload(late, "qW2", 512, 256)
        qW3 = wload(late, "qW3", 512, 256)
        mW1 = wload(late, "mW1", 1536, 256)
        mW2 = wload(late, "mW2", 256, 10)
        AS_nm = late.tile([128, T * K], F32R, tag="AS", name="AS_nm")

        # AS = A @ S: scaled-AT product un-scaled by clamped deg (exact)
        for t in range(T):
            g, j = t // 4, t % 4
            ps = ps_sml(128, K)
            for st in range(4):
                nc.tensor.matmul(
                    ps[:],
                    lhsT=AT[:, (g * 4 + st) * NPG + j * 128 : (g * 4 + st) * NPG + (j + 1) * 128],
                    rhs=S_nm[:, (g * 4 + st) * K : (g * 4 + st + 1) * K],
                    start=(st == 0), stop=(st == 3))
            nc.vector.tensor_scalar(AS_nm[:, t * K : (t + 1) * K], ps[:],
                                    degcl[:, t : t + 1], None, op0=ALU.mult)

        # ---------- h_pool = S^T X, pair-stacked [128 = 2 graphs, .] ----------
        hp_nm = late.tile([128, 2 * 768], F32R, tag="hpn", name="hp_nm")
        hp_fm = late.tile([128, 6 * 256], F32R, tag="hpf", name="hp_fm")
        for h in range(2):
            for L in range(3):
                for gs in range(2):
                    g = h * 2 + gs
                    ps = ps_med(64, 256)
                    for j in range(4):
                        t = g * 4 + j
                        nc.tensor.matmul(
                            ps[:],
                            lhsT=S_nm[:, t * K : (t + 1) * K],
                            rhs=Xr[L][:, t * HID : (t + 1) * HID],
                            start=(j == 0), stop=(j == 3))
                    dst = hp_nm[gs * 64 : gs * 64 + 64,
                                h * 768 + L * 256 : h * 768 + (L + 1) * 256]
                    if gs == 0:
                        nc.vector.tensor_copy(dst, ps[:])
                    else:
                        sh = late.tile([64, 256], F32R, tag="hpsh",
                                       name=_nm("hpsh"), bufs=2)
                        nc.vector.tensor_copy(sh[:], ps[:])
                        nc.sync.dma_start(dst, sh[:])
            for ch in range(6):  # hp_fm via transposes of the pair tile
                tp = ps_med(128, 128, F32R)
                nc.tensor.transpose(
                    tp[:], hp_nm[:, h * 768 + ch * 128 : h * 768 + (ch + 1) * 128],
                    identr[:])
                nc.vector.tensor_copy(
                    hp_fm[:, ch * 256 + h * 128 : ch * 256 + (h + 1) * 128], tp[:])

        # ---------- adj = S^T (A S), pair-stacked; row-normalized ----------
        adjg = late.tile([128, 2 * K], F32, tag="adjg", name="adjg")
        rsum = late.tile([128, 2], F32, tag="rsum", name="rsum")
        adjT = late.tile([128, 2 * 128], F32R, tag="adjT", name="adjT")
        nc.vector.memset(adjT[:].bitcast(F32), 0.0)
        for h in range(2):
            for gs in range(2):
                g = h * 2 + gs
                ps = ps_sml(64, K)
                for j in range(4):
                    t = g * 4 + j
                    nc.tensor.matmul(ps[:],
                                     lhsT=S_nm[:, t * K : (t + 1) * K],
                                     rhs=AS_nm[:, t * K : (t + 1) * K],
                                     start=(j == 0), stop=(j == 3))
                dst = adjg[gs * 64 : gs * 64 + 64, h * K : (h + 1) * K]
                if gs == 0:
                    nc.vector.tensor_copy(dst, ps[:])
                else:
                    sh = late.tile([64, K], F32, tag="adsh",
                                   name=_nm("adsh"), bufs=2)
                    nc.vector.tensor_copy(sh[:], ps[:])
                    nc.sync.dma_start(dst, sh[:])
            nc.vector.tensor_reduce(rsum[:, h : h + 1], adjg[:, h * K : (h + 1) * K],
                                    axis=AX.X, op=ALU.add)
            nc.vector.tensor_scalar(rsum[:, h : h + 1], rsum[:, h : h + 1],
                                    1e-9, None, op0=ALU.add)
            nc.vector.reciprocal(rsum[:, h : h + 1], rsum[:, h : h + 1])
            nc.vector.tensor_scalar(adjg[:, h * K : (h + 1) * K],
                                    adjg[:, h * K : (h + 1) * K],
                                    rsum[:, h : h + 1], None, op0=ALU.mult)
            # transpose each graph's [64,64] block onto the block diagonal
            # (transpose outputs must land at PSUM partition 0; odd block is
            # partition-shifted into place with a small SBUF->SBUF DMA)
            for gs in range(2):
                tp = ps_sml(128, K)
                nc.tensor.transpose(
                    tp[0:64, :],
                    adjg[gs * 64 : gs * 64 + 64, h * K : (h + 1) * K],
                    ident[gs * 64 : gs * 64 + 64, gs * 64 : gs * 64 + 64]
                    if gs else ident[0:64, 0:64])
                if gs == 0:
                    nc.vector.tensor_copy(adjT[0:64, h * 128 : h * 128 + 64],
                                          tp[0:64, :])
                else:
                    sb = late.tile([64, K], F32R, tag="adjsh", name=_nm("adjsh"),
                                   bufs=2)
                    nc.vector.tensor_copy(sb[:], tp[0:64, :])
                    nc.sync.dma_start(
                        adjT[64:128, h * 128 + 64 : h * 128 + 128], sb[:])

        # ---------- pooled sage stack (pair-batched) ----------
        hn1_fm = late.tile([128, 6 * 256], F32R, tag="hn1", name="hn1_fm")
        p1_nm = late.tile([128, 2 * 256], F32R, tag="p1n", name="p1_nm")
        p1_fm = late.tile([128, 2 * 256], F32R, tag="p1f", name="p1_fm")
        hn2_fm = late.tile([128, 2 * 256], F32R, tag="hn2", name="hn2_fm")
        p2_nm = late.tile([128, 2 * 256], F32R, tag="p2n", name="p2_nm")
        p2_fm = late.tile([128, 2 * 256], F32R, tag="p2f", name="p2_fm")
        hn3_fm = late.tile([128, 2 * 256], F32R, tag="hn3", name="hn3_fm")
        p3_fm = late.tile([128, 2 * 256], F32R, tag="p3f", name="p3_fm")

        def pool_hn(x_nm, xw, out_t):
            # out[d, u-pair] = sum_{v-pair} x_nm[v, d] * adjT_bd[v, u]
            for h in range(2):
                for ch in range(xw // 128):
                    tp = ps_sml(128, 128)
                    nc.tensor.matmul(
                        tp[:],
                        lhsT=x_nm[:, h * xw + ch * 128 : h * xw + (ch + 1) * 128],
                        rhs=adjT[:, h * 128 : (h + 1) * 128],
                        start=True, stop=True)
                    nc.vector.tensor_copy(
                        out_t[:, ch * 256 + h * 128 : ch * 256 + (h + 1) * 128], tp[:])

        def pool_lin(xf, hf, Din, Wsb, bccol, rbias, relu, outf, outn):
            nch = Din // 256
            for co in range(2):
                ps = ps_med(128, 256)
                ki = 0
                for src in (xf, hf):
                    for ch in range(nch):
                        nc.tensor.matmul(
                            ps[:],
                            lhsT=Wsb[:, ki * 256 + co * 128 : ki * 256 + co * 128 + 128],
                            rhs=src[:, ch * 256 : (ch + 1) * 256],
                            start=(ki == 0), stop=(ki == 2 * nch - 1))
                        ki += 1
                nc.scalar.activation(
                    outf[:, co * 256 : (co + 1) * 256],
                    ps[:], AF.Relu if relu else AF.Identity,
                    bias=bcol[:, bccol + co : bccol + co + 1])
            if outn is not None:
                for h in range(2):
                    ps = ps_med(128, 256)
                    ki = 0
                    for src in (xf, hf):
                        for ch in range(nch):
                            nc.tensor.matmul(
                                ps[:],
                                lhsT=src[:, ch * 256 + h * 128 : ch * 256 + (h + 1) * 128],
                                rhs=Wsb[:, ki * 256 : (ki + 1) * 256],
                                start=(ki == 0), stop=False)
                            ki += 1
                    nc.tensor.matmul(ps[:], lhsT=ones_at(rbias[0], 128).bitcast(F32),
                                     rhs=rrow(rbias, 256).bitcast(F32),
                                     start=False, stop=True)
                    nc.vector.tensor_scalar(outn[:, h * 256 : (h + 1) * 256], ps[:],
                                            0.0, None, op0=ALU.max)

        pool_hn(hp_nm, 768, hn1_fm)
        pool_lin(hp_fm, hn1_fm, 1536, qW1, BC_QB1, R_QB1, True, p1_fm, p1_nm)
        pool_hn(p1_nm, 256, hn2_fm)
        pool_lin(p1_fm, hn2_fm, 512, qW2, BC_QB2, R_QB2, True, p2_fm, p2_nm)
        pool_hn(p2_nm, 256, hn3_fm)
        pool_lin(p2_fm, hn3_fm, 512, qW3, BC_QB3, R_QB3, False, p3_fm, None)
        for L, pf in enumerate((p1_fm, p2_fm, p3_fm)):
            for co in range(2):
                for g in range(G):
                    nc.vector.tensor_reduce(
                        out_fm[:, (6 + L * 2 + co) * G + g : (6 + L * 2 + co) * G + g + 1],
                        pf[:, co * 256 + g * K : co * 256 + (g + 1) * K],
                        axis=AX.X, op=ALU.max)

        # ---------- final MLP ----------
        for co in range(2):
            ps = ps_sml(128, G)
            for k in range(12):
                nc.tensor.matmul(
                    ps[:], lhsT=mW1[:, k * 256 + co * 128 : k * 256 + co * 128 + 128],
                    rhs=out_fm[:, k * G : (k + 1) * G],
                    start=(k == 0), stop=(k == 11))
            nc.scalar.activation(y_sb[:, co * G : (co + 1) * G], ps[:], AF.Identity,
                                 bias=bcol[:, BC_MB1 + co : BC_MB1 + co + 1])
        zps = ps_sml(10, G)
        for ci in range(2):
            nc.tensor.matmul(zps[:], lhsT=mW2[:, ci * 10 : (ci + 1) * 10],
                             rhs=y_sb[:, ci * G : (ci + 1) * G],
                             start=(ci == 0), stop=(ci == 1))
        nc.scalar.activation(z_sb[:], zps[:], AF.Identity,
                             bias=bcol[0:10, BC_MB2 : BC_MB2 + 1])
        nc.sync.dma_start(yp_d[:], z_sb[:])

    nc.compile()
    return nc


# ---------------------------------------------------------------------------
# host side
# ---------------------------------------------------------------------------

def _pack_bcol(b):
    bc = np.zeros((128, BC_N), np.float32)
    for off, k in ((BC_B1, "b1"), (BC_B2, "b2"), (BC_B3, "b3"), (BC_AB1, "ab1"),
                   (BC_AB2, "ab2"), (BC_AB3, "ab3"), (BC_QB1, "qb1"),
                   (BC_QB2, "qb2"), (BC_QB3, "qb3"), (BC_MB1, "mb1")):
        v = np.asarray(b[k], np.float32)
        bc[:, off : off + v.size // 128] = v.reshape(-1, 128).T
    mb2 = np.asarray(b["mb2"], np.float32)
    bc[: mb2.size, BC_MB2] = mb2
    return bc


def tf32_round(v):
    u = np.ascontiguousarray(np.asarray(v, np.float32)).view(np.uint32).copy()
    u &= np.uint32(0xFFFFE000)
    return u.view(np.float32)


def _pack_rows(b, pb_lc):
    r = np.zeros((65, ROWS_W), np.float32)
    for p in (0, 32, 64):
        r[p, 0:512] = 1.0
    for (p, off), k in ((R_QB1, "qb1"), (R_QB2, "qb2"), (R_QB3, "qb3")):
        r[p, off : off + 256] = b[k]
    p, off = R_PB
    r[p, off : off + 256] = pb_lc
    return tf32_round(r)


def _edge_tables(edge_src, edge_dst, core):
    """Dedup'd (dst, count) tables per (src-tile, partition) for one core."""
    lo, hi = core * NLOC, (core + 1) * NLOC
    m = (edge_dst >= lo) & (edge_dst < hi)
    src = edge_src[m].astype(np.int64)
    dst = edge_dst[m].astype(np.int64)
    gg = dst // NPG
    if not np.array_equal(src // NPG, gg):
        raise ValueError("cross-graph edges break graph-parallel sharding")
    gl = gg - core * G
    sl = src - gg * NPG
    dl = dst - gg * NPG
    t = gl * 4 + sl // 128
    p = sl % 128
    key = (t * 128 + p) * NPG + dl
    uk, cnt = np.unique(key, return_counts=True)
    rows = uk // NPG
    cols = uk % NPG
    nrow = np.bincount(rows, minlength=T * 128)
    if nrow.max() > NI:
        raise ValueError(f"out-degree {nrow.max()} exceeds NI={NI}")
    starts = np.zeros(T * 128, np.int64)
    np.cumsum(nrow[:-1], out=starts[1:])
    pos = np.arange(uk.size) - starts[rows]
    at_idx = np.full((128, T * NI), -1, np.int16)
    at_val = np.zeros((128, T * NI), np.float32)
    pr = (rows % 128).astype(np.int64)
    tr = (rows // 128).astype(np.int64)
    at_idx[pr, tr * NI + pos] = cols.astype(np.int16)
    at_val[pr, tr * NI + pos] = cnt.astype(np.float32)
    return at_idx, at_val.astype(ml_dtypes.bfloat16)


_CACHE = {}
TRACE = False


def prepare_in_maps(inputs):
    f32 = lambda x: np.ascontiguousarray(np.asarray(x, np.float32))
    feat = f32(inputs["feat"])
    edge_src = np.asarray(inputs["edge_src"])
    edge_dst = np.asarray(inputs["edge_dst"])
    W = {k: tf32_round(inputs[k]) for k in
         ("W1", "W2", "W3", "aW1", "aW2", "aW3", "pW", "qW1", "qW2", "qW3",
          "mW1", "mW2")}
    b = {k: f32(inputs[k]) for k in
         ("b1", "b2", "b3", "ab1", "ab2", "ab3", "pb", "qb1", "qb2", "qb3",
          "mb1", "mb2")}
    ident = np.eye(128, dtype=np.float32)
    bcol = _pack_bcol(b)

    in_maps = []
    for c in range(NCORES):
        fs = feat[c * NLOC : (c + 1) * NLOC]
        feat_nm = np.ascontiguousarray(
            fs.reshape(T, 128, IN).transpose(1, 0, 2).reshape(128, T * IN))
        featT = np.ascontiguousarray(fs.T)
        at_idx, at_val = _edge_tables(edge_src, edge_dst, c)
        pW_lc = np.ascontiguousarray(W["pW"][:, c * G * K : (c + 1) * G * K])
        pb_lc = np.ascontiguousarray(b["pb"][c * G * K : (c + 1) * G * K])
        in_maps.append({
            "featT": tf32_round(featT), "feat_nm": tf32_round(feat_nm),
            "at_idx": at_idx, "at_val": at_val,
            "bcol": bcol, "rows2": _pack_rows(b, pb_lc),
            "ident": ident, "identr": ident,
            "W1": W["W1"], "W2": W["W2"], "W3": W["W3"],
            "aW1": W["aW1"], "aW2": W["aW2"], "aW3": W["aW3"],
            "pWl": pW_lc, "qW1": W["qW1"], "qW2": W["qW2"], "qW3": W["qW3"],
            "mW1": W["mW1"], "mW2": W["mW2"],
        })
    return in_maps


def kernel(**inputs):
    if "nc" not in _CACHE:
        _CACHE["nc"] = build_module()
    nc = _CACHE["nc"]
    in_maps = prepare_in_maps(inputs)
    res = run_bass_kernel_spmd(nc, in_maps, core_ids=list(range(NCORES)),
                               trace=TRACE)
    _CACHE["last_res"] = res
    out = np.zeros((B, 10), np.float32)
    for c in range(NCORES):
        out[c * G : (c + 1) * G, :] = np.asarray(res.results[c]["yp"]).T
    return out



# revision 5
# speedup vs baseline: 1.4847x; 1.4847x over previous
"""DiffPoolEncoder Trainium2 kernel (v2: fp16 datapath).

Sharding: data parallel by graph. 8 cores x 4 graphs (512 nodes each).
Per core the whole network runs on-device. GraphSage aggregation uses dense
per-graph A^T tiles pre-scaled on host (cnt/deg, exact in fp16 since deg=16)
and DMA'd directly. The whole activation/weight datapath is fp16 (same 10
mantissa bits as tf32; accumulation stays fp32 in PSUM), which halves SBUF
and DMA so nothing spills to DRAM and all weights stay resident. PSUM
evacuations are round-robined across the Activation/DVE/Pool engines.
"""

import sys

for _p in ("/opt/trn_rl_repo",):
    if _p not in sys.path:
        sys.path.append(_p)

import numpy as np
from contextlib import ExitStack

import concourse.bass as bass
import concourse.mybir as mybir
import concourse.tile as tile
from concourse import bacc
from concourse.bass_utils import run_bass_kernel_spmd

F32 = mybir.dt.float32
F16 = mybir.dt.float16
AF = mybir.ActivationFunctionType
ALU = mybir.AluOpType
AX = mybir.AxisListType

NCORES = 8
B = 32
NPG = 512
G = 4            # graphs per core
T = 16           # node tiles per core (4 per graph)
NLOC = 2048      # nodes per core
K = 64           # clusters per graph
IN = 128
HID = 256

# bcol column layout (each 128-chunk of a bias vector is one column)
BC_B1, BC_B2, BC_B3 = 0, 2, 4
BC_AB1, BC_AB2 = 6, 8
BC_AB3 = 10          # 16 cols
BC_QB1, BC_QB2, BC_QB3 = 26, 28, 30
BC_MB1, BC_MB2 = 32, 34
BC_N = 35

# rows2 [65, 1024] f16: rows at matmul base partitions {0, 32, 64};
# ones[0:512] replicated at each used partition (matmul needs equal bases).
R_QB1 = (0, 512)
R_QB2, R_QB3 = (64, 512), (64, 768)
R_PB = (32, 512)     # 256 (per-core pW bias slice)
ROWS_W = 1024


def build_module():
    nc = bacc.Bacc("TRN2", target_bir_lowering=False)

    # ---------------- DRAM I/O ----------------
    featT_d = nc.dram_tensor("featT", [128, NLOC], F16, kind="ExternalInput")
    featnm_d = nc.dram_tensor("feat_nm", [128, T * IN], F16, kind="ExternalInput")
    at_d = nc.dram_tensor("at_dense", [128, T * NPG], F16, kind="ExternalInput")
    degc_d = nc.dram_tensor("degc", [128, T], F32, kind="ExternalInput")
    bcol_d = nc.dram_tensor("bcol", [128, BC_N], F32, kind="ExternalInput")
    rows_d = nc.dram_tensor("rows2", [65, ROWS_W], F16, kind="ExternalInput")
    identr_d = nc.dram_tensor("identr", [128, 128], F16, kind="ExternalInput")
    w_d = {}
    for name, fi, fo in [
        ("W1", 256, 256), ("W2", 512, 256), ("W3", 512, 256),
        ("aW1", 256, 256), ("aW2", 512, 256), ("aW3", 512, 2048),
        ("pWl", 2560, 256), ("qW1", 1536, 256), ("qW2", 512, 256),
        ("qW3", 512, 256), ("mW1", 1536, 256), ("mW2", 256, 10),
    ]:
        w_d[name] = nc.dram_tensor(name, [fi, fo], F16, kind="ExternalInput")
    yp_d = nc.dram_tensor("yp", [10, G], F32, kind="ExternalOutput")

    with tile.TileContext(nc) as tc, ExitStack() as ex, \
            nc.allow_low_precision(reason="fp16 keeps tf32's 10 mantissa bits; accumulation stays fp32 in PSUM"):
        persist = ex.enter_context(tc.tile_pool(name="persist", bufs=1))
        # PSUM: 8 banks. One tag per pool so slot count == bank count.
        ps_p = ex.enter_context(tc.tile_pool(name="psP", bufs=4, space="PSUM"))
        lg_p = ex.enter_context(tc.tile_pool(name="psL", bufs=1, space="PSUM"))
        pm_p = ex.enter_context(tc.tile_pool(name="psM", bufs=2, space="PSUM"))
        pl_p = ex.enter_context(tc.tile_pool(name="psS", bufs=1, space="PSUM"))

        uid = [0]

        def _nm(pfx):
            uid[0] += 1
            return f"{pfx}{uid[0]}"

        def ps_big(dt=F32):
            return ps_p.tile([128, 512], dt, tag="ps", name=_nm("ps"))

        def ps_med(p, f, dt=F32):
            return pm_p.tile([p, f], dt, tag="pm", name=_nm("pm"))

        def ps_sml(p, f, dt=F32):
            return pl_p.tile([p, f], dt, tag="pl", name=_nm("pl"))

        def wload(pool, name, fi, fo):
            kk = fi // 128
            sb = pool.tile([128, kk * fo], F16, tag=name, name=name)
            nc.sync.dma_start(
                sb[:].rearrange("p (k f) -> p k f", k=kk, f=fo),
                w_d[name][:, :].rearrange("(k p) f -> p k f", p=128),
            )
            return sb

        # ---------- persistent small tensors ----------
        identr = persist.tile([128, 128], F16)
        rows2 = persist.tile([65, ROWS_W], F16)
        bcol = persist.tile([128, BC_N], F32)
        degc = persist.tile([128, T], F32)
        S_nm = persist.tile([128, T * K], F16)
        lgs_nm = persist.tile([128, T * K], F16)
        out_fm = persist.tile([128, 12 * G], F16)  # readout maxes, col=ch*G+g
        nmax = persist.tile([128, 2], F32)
        sumx = persist.tile([128, 2], F32)
        y_sb = persist.tile([128, 2 * G], F16)
        z_sb = persist.tile([10, G], F32)

        def ones_at(p, n):
            return rows2[p : p + 1, 0:n]

        def rrow(ro, n):
            p, off = ro
            return rows2[p : p + 1, off : off + n]

        # ---------- pools (opened in LIFO close order; fnmp closes first) ----------
        hres = ex.enter_context(tc.tile_pool(name="hres", bufs=1))
        xfm_p = ex.enter_context(tc.tile_pool(name="xfm", bufs=2))
        xnm_p = ex.enter_context(tc.tile_pool(name="xnm", bufs=2))
        agg_p = ex.enter_context(tc.tile_pool(name="aggfm", bufs=2))
        mid_p = ex.enter_context(tc.tile_pool(name="midp", bufs=1))
        fnm_p = ExitStack()
        fnmpool = fnm_p.enter_context(tc.tile_pool(name="fnmp", bufs=1))

        # ---------- input DMAs (issue order == priority order) ----------
        AT = persist.tile([128, T * NPG], F16, tag="AT", name="AT")
        featnm = fnmpool.tile([128, T * IN], F16, tag="featnm")
        featT = fnmpool.tile([128, NLOC], F16, tag="featT")
        nc.sync.dma_start(featnm[:], featnm_d[:])
        for g in range(G):
            sl = slice(g * 4 * NPG, (g + 1) * 4 * NPG)
            nc.sync.dma_start(AT[:, sl], at_d[:, sl])
        nc.sync.dma_start(featT[:], featT_d[:])
        W1 = wload(persist, "W1", 256, 256)
        aW1 = wload(persist, "aW1", 256, 256)
        nc.sync.dma_start(identr[:], identr_d[:])
        nc.sync.dma_start(rows2[:], rows_d[:])
        nc.sync.dma_start(bcol[:], bcol_d[:])
        nc.sync.dma_start(degc[:], degc_d[:])
        W2 = wload(persist, "W2", 512, 256)
        aW2 = wload(persist, "aW2", 512, 256)
        W3 = wload(persist, "W3", 512, 256)
        aW3 = persist.tile([128, 4 * 2048], F16, tag="aW3", name="aW3")
        aW3v = aW3[:].rearrange("p (k f) -> p k f", k=4, f=2048)
        for q in range(4):
            nc.sync.dma_start(
                aW3v[:, :, q * 512 : (q + 1) * 512],
                w_d["aW3"][:, q * 512 : (q + 1) * 512].rearrange(
                    "(k p) f -> p k f", p=128))
        pWl = wload(persist, "pWl", 2560, 256)
        qW1 = wload(persist, "qW1", 1536, 256)
        qW2 = wload(persist, "qW2", 512, 256)
        qW3 = wload(persist, "qW3", 512, 256)
        mW1 = wload(persist, "mW1", 1536, 256)
        mW2 = wload(persist, "mW2", 256, 10)

        # ---------- activation tiles ----------
        h1n = hres.tile([128, T * HID], F16, tag="h1n", name="h1n")
        h2n = hres.tile([128, T * HID], F16, tag="h2n", name="h2n")
        h3n = hres.tile([128, T * HID], F16, tag="h3n", name="h3n")
        a1f = hres.tile([128, 2 * NLOC], F16, tag="a1f", name="a1f")
        a2f = hres.tile([128, 2 * NLOC], F16, tag="a2f", name="a2f")

        # ---------- PSUM evacuation, round-robin across engines ----------
        rr = [0]

        def evac(dst, src, bias=None, relu=False, w=(1, 1)):
            """dst = act(src + bias); engines weighted (Act, DVE).
            GPSIMD cannot read PSUM, so only these two can evacuate."""
            sel = rr[0] % (w[0] + w[1])
            rr[0] += 1
            if sel < w[0]:
                nc.scalar.activation(dst, src, AF.Relu if relu else AF.Identity,
                                     bias=bias if bias is not None else 0.0)
            else:
                if bias is None and not relu:
                    nc.vector.tensor_copy(dst, src)
                elif relu:
                    nc.vector.tensor_scalar(dst, src,
                                            bias if bias is not None else 0.0,
                                            0.0, op0=ALU.add, op1=ALU.max)
                else:
                    nc.vector.tensor_scalar(dst, src, bias, None, op0=ALU.add)

        # ---------- emit helpers ----------
        def emit_agg(x_nm, D, out_t):
            """out_t[d, n] (feature-major) = sum_s x_nm[s, d] * AT[s, n]."""
            for g in range(G):
                for ch in range(D // 128):
                    ps = ps_big()
                    for st in range(4):
                        t = g * 4 + st
                        nc.tensor.matmul(
                            ps[:],
                            lhsT=x_nm[:, t * D + ch * 128 : t * D + ch * 128 + 128],
                            rhs=AT[:, t * NPG : (t + 1) * NPG],
                            start=(st == 0), stop=(st == 3))
                    evac(out_t[:, ch * NLOC + g * NPG : ch * NLOC + (g + 1) * NPG],
                         ps[:])

        def emit_lin_fm(x_fm, a_fm, Din, Dout, Wsb, bccol, relu, out_t):
            nk = Din // 128
            for co in range(Dout // 128):
                for nb in range(4):
                    ps = ps_big()
                    ki = 0
                    for src in (x_fm, a_fm):
                        for ci in range(nk):
                            nc.tensor.matmul(
                                ps[:],
                                lhsT=Wsb[:, ki * Dout + co * 128 : ki * Dout + co * 128 + 128],
                                rhs=src[:, ci * NLOC + nb * 512 : ci * NLOC + (nb + 1) * 512],
                                start=(ki == 0), stop=(ki == 2 * nk - 1))
                            ki += 1
                    evac(out_t[:, co * NLOC + nb * 512 : co * NLOC + (nb + 1) * 512],
                         ps[:], bias=bcol[:, bccol + co : bccol + co + 1], relu=relu)

        def emit_nm_T(x_fm, out_nm):
            # node-major via PE transposes of the (already relu'd) fm tensor;
            # 4 transposed blocks share one PSUM bank -> single 512-wide evac.
            for t2 in range(0, T, 2):
                tp = ps_big(F16)
                for i, (t, ch) in enumerate(
                        ((t2, 0), (t2, 1), (t2 + 1, 0), (t2 + 1, 1))):
                    nc.tensor.matmul(
                        tp[:, i * 128 : (i + 1) * 128],
                        lhsT=x_fm[:, ch * NLOC + t * 128 : (t + 1) * 128 + ch * NLOC],
                        rhs=identr[:], is_transpose=True,
                        start=True, stop=True, skip_group_check=True)
                evac(out_nm[:, t2 * HID : (t2 + 2) * HID], tp[:])

        def emit_out1(x_fm, ch0):
            for ci in range(2):
                for g in range(G):
                    nc.vector.tensor_reduce(
                        out_fm[:, (ch0 + ci) * G + g : (ch0 + ci) * G + g + 1],
                        x_fm[:, ci * NLOC + g * NPG : ci * NLOC + (g + 1) * NPG],
                        axis=AX.X, op=ALU.max)

        # ---------- GC stacks ----------
        aggfeat = agg_p.tile([128, NLOC], F16, tag="agg", name="aggfeat")
        emit_agg(featnm, IN, aggfeat)

        h1f = xfm_p.tile([128, 2 * NLOC], F16, tag="xfm", name="h1f")
        emit_lin_fm(featT, aggfeat, 128, 256, W1, BC_B1, True, h1f)
        emit_nm_T(h1f, h1n)
        emit_out1(h1f, 0)

        a1n = xnm_p.tile([128, T * HID], F16, tag="xnm", name="a1n")
        emit_lin_fm(featT, aggfeat, 128, 256, aW1, BC_AB1, True, a1f)
        emit_nm_T(a1f, a1n)
        fnm_p.close()

        aggh1 = agg_p.tile([128, 2 * NLOC], F16, tag="agg2", name="aggh1")
        emit_agg(h1n, HID, aggh1)

        agga1 = agg_p.tile([128, 2 * NLOC], F16, tag="agg2", name="agga1")
        emit_agg(a1n, HID, agga1)

        h2f = xfm_p.tile([128, 2 * NLOC], F16, tag="xfm", name="h2f")
        emit_lin_fm(h1f, aggh1, 256, 256, W2, BC_B2, True, h2f)
        emit_nm_T(h2f, h2n)
        emit_out1(h2f, 2)

        a2n = xnm_p.tile([128, T * HID], F16, tag="xnm", name="a2n")
        emit_lin_fm(a1f, agga1, 256, 256, aW2, BC_AB2, True, a2f)
        emit_nm_T(a2f, a2n)

        aggh2 = agg_p.tile([128, 2 * NLOC], F16, tag="agg2", name="aggh2")
        emit_agg(h2n, HID, aggh2)

        agga2 = agg_p.tile([128, 2 * NLOC], F16, tag="agg2", name="agga2")
        emit_agg(a2n, HID, agga2)

        # h3: fm + readout + node-major (resident, no spill)
        h3f = xfm_p.tile([128, 2 * NLOC], F16, tag="xfm", name="h3f")
        emit_lin_fm(h2f, aggh2, 256, 256, W3, BC_B3, False, h3f)
        emit_nm_T(h3f, h3n)
        emit_out1(h3f, 4)

        # ---------- a3 + logits (streamed per graph) ----------
        for g in range(G):
            lps = lg_p.tile([64, 512], F32, tag="lg", name=_nm("lg"))
            for co in range(16):  # a3 = relu(cat(a2, agg_a2) @ aW3 + ab3)
                ps3 = ps_big()
                ki = 0
                for src in (a2f, agga2):
                    for ci in range(2):
                        nc.tensor.matmul(
                            ps3[:],
                            lhsT=aW3[:, ki * 2048 + co * 128 : ki * 2048 + co * 128 + 128],
                            rhs=src[:, ci * NLOC + g * NPG : ci * NLOC + (g + 1) * NPG],
                            start=(ki == 0), stop=(ki == 3))
                        ki += 1
                a3b = mid_p.tile([128, 512], F16, tag="a3buf", name=_nm("a3b"), bufs=3)
                evac(a3b[:], ps3[:], bias=bcol[:, BC_AB3 + co : BC_AB3 + co + 1],
                     relu=True)
                nc.tensor.matmul(
                    lps[:], lhsT=pWl[:, (4 + co) * 256 + g * K : (4 + co) * 256 + g * K + K],
                    rhs=a3b[:], start=(co == 0), stop=False)
            for ci in range(2):  # a1 block of pW
                nc.tensor.matmul(
                    lps[:], lhsT=pWl[:, ci * 256 + g * K : ci * 256 + g * K + K],
                    rhs=a1f[:, ci * NLOC + g * NPG : ci * NLOC + (g + 1) * NPG],
                    start=False, stop=False)
            for ci in range(2):  # a2 block
                nc.tensor.matmul(
                    lps[:], lhsT=pWl[:, (2 + ci) * 256 + g * K : (2 + ci) * 256 + g * K + K],
                    rhs=a2f[:, ci * NLOC + g * NPG : ci * NLOC + (g + 1) * NPG],
                    start=False, stop=False)
            nc.tensor.matmul(lps[:],
                             lhsT=rrow(R_PB, 256)[:, g * K : (g + 1) * K],
                             rhs=ones_at(32, 512), start=False, stop=True)
            lgf = mid_p.tile([64, 512], F16, tag="lgf", name=_nm("lgf"), bufs=1)
            evac(lgf[:], lps[:])
            for j in range(4):  # transpose to node-major
                t = g * 4 + j
                tps = ps_med(128, 64, F16)
                nc.tensor.matmul(tps[:], lhsT=lgf[0:64, j * 128 : (j + 1) * 128],
                                 rhs=identr[0:64, 0:64], is_transpose=True,
                                 start=True, stop=True)
                nc.vector.tensor_copy(lgs_nm[:, t * K : (t + 1) * K], tps[:])

        # masked softmax == per-graph softmax over K columns
        for t in range(T):
            bb = t % 2
            nc.vector.tensor_reduce(nmax[:, bb : bb + 1], lgs_nm[:, t * K : (t + 1) * K],
                                    axis=AX.X, op=ALU.max, negate=True)
            nc.scalar.activation(S_nm[:, t * K : (t + 1) * K],
                                 lgs_nm[:, t * K : (t + 1) * K], AF.Exp,
                                 bias=nmax[:, bb : bb + 1],
                                 accum_out=sumx[:, bb : bb + 1])
            nc.vector.reciprocal(sumx[:, bb : bb + 1], sumx[:, bb : bb + 1])
            nc.vector.tensor_scalar(S_nm[:, t * K : (t + 1) * K],
                                    S_nm[:, t * K : (t + 1) * K],
                                    sumx[:, bb : bb + 1], None, op0=ALU.mult)

        # ---------- late pool: pooled stage ----------
        late = ex.enter_context(tc.tile_pool(name="late", bufs=1))
        Xr = [h1n, h2n, h3n]
        AS_nm = late.tile([128, T * K], F16, tag="AS", name="AS_nm")

        # AS = A @ S: scaled-AT product un-scaled by clamped deg (exact)
        for t in range(T):
            g, j = t // 4, t % 4
            ps = ps_sml(128, K)
            for st in range(4):
                nc.tensor.matmul(
                    ps[:],
                    lhsT=AT[:, (g * 4 + st) * NPG + j * 128 : (g * 4 + st) * NPG + (j + 1) * 128],
                    rhs=S_nm[:, (g * 4 + st) * K : (g * 4 + st + 1) * K],
                    start=(st == 0), stop=(st == 3))
            nc.vector.tensor_scalar(AS_nm[:, t * K : (t + 1) * K], ps[:],
                                    degc[:, t : t + 1], None, op0=ALU.mult)

        # ---------- h_pool = S^T X, pair-stacked [128 = 2 graphs, .] ----------
        hp_nm = late.tile([128, 2 * 768], F16, tag="hpn", name="hp_nm")
        hp_fm = late.tile([128, 6 * 256], F16, tag="hpf", name="hp_fm")
        for h in range(2):
            for L in range(3):
                for gs in range(2):
                    g = h * 2 + gs
                    ps = ps_med(64, 256)
                    for j in range(4):
                        t = g * 4 + j
                        nc.tensor.matmul(
                            ps[:],
                            lhsT=S_nm[:, t * K : (t + 1) * K],
                            rhs=Xr[L][:, t * HID : (t + 1) * HID],
                            start=(j == 0), stop=(j == 3))
                    dst = hp_nm[gs * 64 : gs * 64 + 64,
                                h * 768 + L * 256 : h * 768 + (L + 1) * 256]
                    if gs == 0:
                        nc.vector.tensor_copy(dst, ps[:])
                    else:
                        sh = late.tile([64, 256], F16, tag="hpsh",
                                       name=_nm("hpsh"), bufs=2)
                        nc.vector.tensor_copy(sh[:], ps[:])
                        nc.sync.dma_start(dst, sh[:])
            for ch in range(6):  # hp_fm via transposes of the pair tile
                tp = ps_med(128, 128, F16)
                nc.tensor.matmul(
                    tp[:], lhsT=hp_nm[:, h * 768 + ch * 128 : h * 768 + (ch + 1) * 128],
                    rhs=identr[:], is_transpose=True, start=True, stop=True)
                nc.vector.tensor_copy(
                    hp_fm[:, ch * 256 + h * 128 : ch * 256 + (h + 1) * 128], tp[:])

        # ---------- adj = S^T (A S), pair-stacked; row-normalized ----------
        adjg = late.tile([128, 2 * K], F16, tag="adjg", name="adjg")
        rsum = late.tile([128, 2], F32, tag="rsum", name="rsum")
        adjT = late.tile([128, 2 * 128], F16, tag="adjT", name="adjT")
        nc.vector.memset(adjT[:], 0.0)
        for h in range(2):
            for gs in range(2):
                g = h * 2 + gs
                ps = ps_sml(64, K)
                for j in range(4):
                    t = g * 4 + j
                    nc.tensor.matmul(ps[:],
                                     lhsT=S_nm[:, t * K : (t + 1) * K],
                                     rhs=AS_nm[:, t * K : (t + 1) * K],
                                     start=(j == 0), stop=(j == 3))
                dst = adjg[gs * 64 : gs * 64 + 64, h * K : (h + 1) * K]
                if gs == 0:
                    nc.vector.tensor_copy(dst, ps[:])
                else:
                    sh = late.tile([64, K], F16, tag="adsh",
                                   name=_nm("adsh"), bufs=2)
                    nc.vector.tensor_copy(sh[:], ps[:])
                    nc.sync.dma_start(dst, sh[:])
            nc.vector.tensor_reduce(rsum[:, h : h + 1], adjg[:, h * K : (h + 1) * K],
                                    axis=AX.X, op=ALU.add)
            nc.vector.tensor_scalar(rsum[:, h : h + 1], rsum[:, h : h + 1],
                                    1e-9, None, op0=ALU.add)
            nc.vector.reciprocal(rsum[:, h : h + 1], rsum[:, h : h + 1])
            nc.vector.tensor_scalar(adjg[:, h * K : (h + 1) * K],
                                    adjg[:, h * K : (h + 1) * K],
                                    rsum[:, h : h + 1], None, op0=ALU.mult)
            # transpose each graph's [64,64] block onto the block diagonal
            # (transpose outputs must land at PSUM partition 0; odd block is
            # partition-shifted into place with a small SBUF->SBUF DMA)
            for gs in range(2):
                tp = ps_sml(128, K, F16)
                nc.tensor.matmul(
                    tp[0:64, :],
                    lhsT=adjg[gs * 64 : gs * 64 + 64, h * K : (h + 1) * K],
                    rhs=identr[gs * 64 : gs * 64 + 64, gs * 64 : gs * 64 + 64]
                    if gs else identr[0:64, 0:64],
                    is_transpose=True, start=True, stop=True)
                if gs == 0:
                    nc.vector.tensor_copy(adjT[0:64, h * 128 : h * 128 + 64],
                                          tp[0:64, :])
                else:
                    sb = late.tile([64, K], F16, tag="adjsh", name=_nm("adjsh"),
                                   bufs=2)
                    nc.vector.tensor_copy(sb[:], tp[0:64, :])
                    nc.sync.dma_start(
                        adjT[64:128, h * 128 + 64 : h * 128 + 128], sb[:])

        # ---------- pooled sage stack (pair-batched) ----------
        hn1_fm = late.tile([128, 6 * 256], F16, tag="hn1", name="hn1_fm")
        p1_nm = late.tile([128, 2 * 256], F16, tag="p1n", name="p1_nm")
        p1_fm = late.tile([128, 2 * 256], F16, tag="p1f", name="p1_fm")
        hn2_fm = late.tile([128, 2 * 256], F16, tag="hn2", name="hn2_fm")
        p2_nm = late.tile([128, 2 * 256], F16, tag="p2n", name="p2_nm")
        p2_fm = late.tile([128, 2 * 256], F16, tag="p2f", name="p2_fm")
        hn3_fm = late.tile([128, 2 * 256], F16, tag="hn3", name="hn3_fm")
        p3_fm = late.tile([128, 2 * 256], F16, tag="p3f", name="p3_fm")

        def pool_hn(x_nm, xw, out_t):
            # out[d, u-pair] = sum_{v-pair} x_nm[v, d] * adjT_bd[v, u]
            for h in range(2):
                for ch in range(xw // 128):
                    tp = ps_sml(128, 128)
                    nc.tensor.matmul(
                        tp[:],
                        lhsT=x_nm[:, h * xw + ch * 128 : h * xw + (ch + 1) * 128],
                        rhs=adjT[:, h * 128 : (h + 1) * 128],
                        start=True, stop=True)
                    nc.vector.tensor_copy(
                        out_t[:, ch * 256 + h * 128 : ch * 256 + (h + 1) * 128], tp[:])

        def pool_lin(xf, hf, Din, Wsb, bccol, rbias, relu, outf, outn):
            nch = Din // 256
            for co in range(2):
                ps = ps_med(128, 256)
                ki = 0
                for src in (xf, hf):
                    for ch in range(nch):
                        nc.tensor.matmul(
                            ps[:],
                            lhsT=Wsb[:, ki * 256 + co * 128 : ki * 256 + co * 128 + 128],
                            rhs=src[:, ch * 256 : (ch + 1) * 256],
                            start=(ki == 0), stop=(ki == 2 * nch - 1))
                        ki += 1
                nc.scalar.activation(
                    outf[:, co * 256 : (co + 1) * 256],
                    ps[:], AF.Relu if relu else AF.Identity,
                    bias=bcol[:, bccol + co : bccol + co + 1])
            if outn is not None:
                for h in range(2):
                    ps = ps_med(128, 256)
                    ki = 0
                    for src in (xf, hf):
                        for ch in range(nch):
                            nc.tensor.matmul(
                                ps[:],
                                lhsT=src[:, ch * 256 + h * 128 : ch * 256 + (h + 1) * 128],
                                rhs=Wsb[:, ki * 256 : (ki + 1) * 256],
                                start=(ki == 0), stop=False)
                            ki += 1
                    nc.tensor.matmul(ps[:], lhsT=ones_at(rbias[0], 128),
                                     rhs=rrow(rbias, 256),
                                     start=False, stop=True)
                    nc.vector.tensor_scalar(outn[:, h * 256 : (h + 1) * 256], ps[:],
                                            0.0, None, op0=ALU.max)

        pool_hn(hp_nm, 768, hn1_fm)
        pool_lin(hp_fm, hn1_fm, 1536, qW1, BC_QB1, R_QB1, True, p1_fm, p1_nm)
        pool_hn(p1_nm, 256, hn2_fm)
        pool_lin(p1_fm, hn2_fm, 512, qW2, BC_QB2, R_QB2, True, p2_fm, p2_nm)
        pool_hn(p2_nm, 256, hn3_fm)
        pool_lin(p2_fm, hn3_fm, 512, qW3, BC_QB3, R_QB3, False, p3_fm, None)
        for L, pf in enumerate((p1_fm, p2_fm, p3_fm)):
            for co in range(2):
                for g in range(G):
                    nc.vector.tensor_reduce(
                        out_fm[:, (6 + L * 2 + co) * G + g : (6 + L * 2 + co) * G + g + 1],
                        pf[:, co * 256 + g * K : co * 256 + (g + 1) * K],
                        axis=AX.X, op=ALU.max)

        # ---------- final MLP ----------
        for co in range(2):
            ps = ps_sml(128, G)
            for k in range(12):
                nc.tensor.matmul(
                    ps[:], lhsT=mW1[:, k * 256 + co * 128 : k * 256 + co * 128 + 128],
                    rhs=out_fm[:, k * G : (k + 1) * G],
                    start=(k == 0), stop=(k == 11))
            nc.scalar.activation(y_sb[:, co * G : (co + 1) * G], ps[:], AF.Identity,
                                 bias=bcol[:, BC_MB1 + co : BC_MB1 + co + 1])
        zps = ps_sml(10, G)
        for ci in range(2):
            nc.tensor.matmul(zps[:], lhsT=mW2[:, ci * 10 : (ci + 1) * 10],
                             rhs=y_sb[:, ci * G : (ci + 1) * G],
                             start=(ci == 0), stop=(ci == 1))
        nc.scalar.activation(z_sb[:], zps[:], AF.Identity,
                             bias=bcol[0:10, BC_MB2 : BC_MB2 + 1])
        nc.sync.dma_start(yp_d[:], z_sb[:])

    nc.compile()
    return nc


# ---------------------------------------------------------------------------
# host side
# ---------------------------------------------------------------------------

def _pack_bcol(b):
    bc = np.zeros((128, BC_N), np.float32)
    for off, k in ((BC_B1, "b1"), (BC_B2, "b2"), (BC_B3, "b3"), (BC_AB1, "ab1"),
                   (BC_AB2, "ab2"), (BC_AB3, "ab3"), (BC_QB1, "qb1"),
                   (BC_QB2, "qb2"), (BC_QB3, "qb3"), (BC_MB1, "mb1")):
        v = np.asarray(b[k], np.float32)
        bc[:, off : off + v.size // 128] = v.reshape(-1, 128).T
    mb2 = np.asarray(b["mb2"], np.float32)
    bc[: mb2.size, BC_MB2] = mb2
    return bc


def _pack_rows(b, pb_lc):
    r = np.zeros((65, ROWS_W), np.float32)
    for p in (0, 32, 64):
        r[p, 0:512] = 1.0
    for (p, off), k in ((R_QB1, "qb1"), (R_QB2, "qb2"), (R_QB3, "qb3")):
        r[p, off : off + 256] = b[k]
    p, off = R_PB
    r[p, off : off + 256] = pb_lc
    return r.astype(np.float16)


def _at_dense(edge_src, edge_dst, core):
    """Dense scaled A^T tiles [128, T*NPG] fp16 plus clamped-deg cols."""
    lo, hi = core * NLOC, (core + 1) * NLOC
    m = (edge_dst >= lo) & (edge_dst < hi)
    src = edge_src[m].astype(np.int64)
    dst = edge_dst[m].astype(np.int64)
    gg = dst // NPG
    if not np.array_equal(src // NPG, gg):
        raise ValueError("cross-graph edges break graph-parallel sharding")
    gl = gg - core * G
    sl = src - gg * NPG
    dl = dst - gg * NPG
    t = gl * 4 + sl // 128
    p = sl % 128
    flat = (p * T + t) * NPG + dl
    cnt = np.bincount(flat, minlength=128 * T * NPG).astype(np.float64)
    at = cnt.reshape(128, T * NPG)
    # deg per local node (node-major: node = tt*128 + pp)
    nl = gl * NPG + dl
    deg = np.bincount(nl, minlength=NLOC).astype(np.float64)
    degc = np.maximum(deg, 1.0)
    # scale each AT column (dst d of graph g == local node g*NPG+d)
    colnode = (np.arange(T * NPG) // (4 * NPG)) * NPG + np.arange(T * NPG) % NPG
    at = at / degc[colnode][None, :]
    degc_nm = degc.reshape(T, 128).T.astype(np.float32)
    return at.astype(np.float16), np.ascontiguousarray(degc_nm)


_CACHE = {}
TRACE = False


def prepare_in_maps(inputs):
    f16 = lambda x: np.ascontiguousarray(np.asarray(x, np.float32).astype(np.float16))
    feat = np.asarray(inputs["feat"], np.float32)
    edge_src = np.asarray(inputs["edge_src"])
    edge_dst = np.asarray(inputs["edge_dst"])
    W = {k: f16(inputs[k]) for k in
         ("W1", "W2", "W3", "aW1", "aW2", "aW3", "pW", "qW1", "qW2", "qW3",
          "mW1", "mW2")}
    b = {k: np.asarray(inputs[k], np.float32) for k in
         ("b1", "b2", "b3", "ab1", "ab2", "ab3", "pb", "qb1", "qb2", "qb3",
          "mb1", "mb2")}
    identr = np.eye(128, dtype=np.float16)
    bcol = _pack_bcol(b)

    in_maps = []
    for c in range(NCORES):
        fs = feat[c * NLOC : (c + 1) * NLOC]
        feat_nm = np.ascontiguousarray(
            fs.reshape(T, 128, IN).transpose(1, 0, 2).reshape(128, T * IN))
        featT = np.ascontiguousarray(fs.T)
        at, degc = _at_dense(edge_src, edge_dst, c)
        pW_lc = np.ascontiguousarray(W["pW"][:, c * G * K : (c + 1) * G * K])
        pb_lc = np.ascontiguousarray(b["pb"][c * G * K : (c + 1) * G * K])
        in_maps.append({
            "featT": f16(featT), "feat_nm": f16(feat_nm),
            "at_dense": at, "degc": degc,
            "bcol": bcol, "rows2": _pack_rows(b, pb_lc),
            "identr": identr,
            "W1": W["W1"], "W2": W["W2"], "W3": W["W3"],
            "aW1": W["aW1"], "aW2": W["aW2"], "aW3": W["aW3"],
            "pWl": pW_lc, "qW1": W["qW1"], "qW2": W["qW2"], "qW3": W["qW3"],
            "mW1": W["mW1"], "mW2": W["mW2"],
        })
    return in_maps


def kernel(**inputs):
    if "nc" not in _CACHE:
        _CACHE["nc"] = build_module()
    nc = _CACHE["nc"]
    in_maps = prepare_in_maps(inputs)
    res = run_bass_kernel_spmd(nc, in_maps, core_ids=list(range(NCORES)),
                               trace=TRACE)
    _CACHE["last_res"] = res
    out = np.zeros((B, 10), np.float32)
    for c in range(NCORES):
        out[c * G : (c + 1) * G, :] = np.asarray(res.results[c]["yp"]).T
    return out


# revision 14
# speedup vs baseline: 2.0856x; 1.4047x over previous
"""DiffPoolEncoder Trainium2 kernel (v3: fp16 datapath + fp8 DoubleRow a3).

Sharding: data parallel by graph. 8 cores x 4 graphs (512 nodes each).
Per core the whole network runs on-device. GraphSage aggregation uses dense
per-graph A^T tiles pre-scaled on host (cnt/deg, exact in fp16 since deg=16)
and DMA'd directly. The datapath is fp16 (same 10 mantissa bits as tf32;
accumulation stays fp32 in PSUM). The assignment-stack a2 subtree and the
giant a3 GEMM (nodes x 2048 x 512) run in fp8e4m3 with DoubleRow perf mode
(2 contraction rows per PE pass). PSUM evacuations are split across the
Activation/DVE engines; SBUF-side readout reductions run on GpSimd.
"""

import sys

for _p in ("/opt/trn_rl_repo",):
    if _p not in sys.path:
        sys.path.append(_p)

import numpy as np
from contextlib import ExitStack

import concourse.bass as bass
import concourse.mybir as mybir
import concourse.tile as tile
from concourse import bacc
from concourse.bass_utils import run_bass_kernel_spmd

F32 = mybir.dt.float32
F16 = mybir.dt.float16
F8 = mybir.dt.float8e4
DR = mybir.MatmulPerfMode.DoubleRow
AF = mybir.ActivationFunctionType
ALU = mybir.AluOpType
AX = mybir.AxisListType

NCORES = 8
B = 32
NPG = 512
G = 4            # graphs per core
T = 16           # node tiles per core (4 per graph)
NLOC = 2048      # nodes per core
K = 64           # clusters per graph
IN = 128
HID = 256

# bcol column layout (each 128-chunk of a bias vector is one column)
BC_B1, BC_B2, BC_B3 = 0, 2, 4
BC_AB1, BC_AB2 = 6, 8
BC_AB3 = 10          # 16 cols
BC_QB1, BC_QB2, BC_QB3 = 26, 28, 30
BC_MB1, BC_MB2 = 32, 34
BC_N = 35

# rows2 [65, 1024] f16: rows at matmul base partitions {0, 32, 64};
# ones[0:512] replicated at each used partition (matmul needs equal bases).
R_QB1 = (0, 512)
R_QB2, R_QB3 = (64, 512), (64, 768)
R_PB = (32, 512)     # 256 (per-core pW bias slice)
ROWS_W = 1024


def build_module():
    nc = bacc.Bacc("TRN2", target_bir_lowering=False)

    # ---------------- DRAM I/O ----------------
    featT_d = nc.dram_tensor("featT", [128, NLOC], F16, kind="ExternalInput")
    featnm_d = nc.dram_tensor("feat_nm", [128, T * IN], F16, kind="ExternalInput")
    at_d = nc.dram_tensor("at_dense", [128, T * NPG], F16, kind="ExternalInput")
    degc_d = nc.dram_tensor("degc", [128, T], F32, kind="ExternalInput")
    bcol_d = nc.dram_tensor("bcol", [128, BC_N], F32, kind="ExternalInput")
    rows_d = nc.dram_tensor("rows2", [65, ROWS_W], F16, kind="ExternalInput")
    identr_d = nc.dram_tensor("identr", [128, 128], F16, kind="ExternalInput")
    w_d = {}
    for name, fi, fo, dt in [
        ("W1", 256, 256, F16), ("W2", 512, 256, F16), ("W3", 512, 256, F16),
        ("aW1", 256, 256, F16), ("aW2", 512, 256, F16), ("aW3", 512, 2048, F8),
        ("pWa", 512, 256, F16), ("pW3", 2048, 256, F8),
        ("qW1", 1536, 256, F16), ("qW2", 512, 256, F16),
        ("qW3", 512, 256, F16), ("mW1", 1536, 256, F16), ("mW2", 256, 10, F16),
    ]:
        w_d[name] = nc.dram_tensor(name, [fi, fo], dt, kind="ExternalInput")
    yp_d = nc.dram_tensor("yp", [10, G], F32, kind="ExternalOutput")

    with tile.TileContext(nc) as tc, ExitStack() as ex, \
            nc.allow_low_precision(reason="fp16/fp8 datapath; accumulation stays fp32 in PSUM"):
        persist = ex.enter_context(tc.tile_pool(name="persist", bufs=1))
        # PSUM: 8 banks. One tag per pool so slot count == bank count.
        ps_p = ex.enter_context(tc.tile_pool(name="psP", bufs=4, space="PSUM"))
        lg_p = ex.enter_context(tc.tile_pool(name="psL", bufs=1, space="PSUM"))
        pm_p = ex.enter_context(tc.tile_pool(name="psM", bufs=2, space="PSUM"))
        pl_p = ex.enter_context(tc.tile_pool(name="psS", bufs=1, space="PSUM"))

        uid = [0]

        def _nm(pfx):
            uid[0] += 1
            return f"{pfx}{uid[0]}"

        def ps_big(dt=F32):
            return ps_p.tile([128, 512], dt, tag="ps", name=_nm("ps"))

        def ps_med(p, f, dt=F32):
            return pm_p.tile([p, f], dt, tag="pm", name=_nm("pm"))

        def ps_sml(p, f, dt=F32):
            return pl_p.tile([p, f], dt, tag="pl", name=_nm("pl"))

        def wload(pool, name, fi, fo, dt=F16):
            kk = fi // 128
            sb = pool.tile([128, kk * fo], dt, tag=name, name=name)
            nc.sync.dma_start(
                sb[:].rearrange("p (k f) -> p k f", k=kk, f=fo),
                w_d[name][:, :].rearrange("(k p) f -> p k f", p=128),
            )
            return sb

        # ---------- persistent small tensors ----------
        identr = persist.tile([128, 128], F16)
        rows2 = persist.tile([65, ROWS_W], F16)
        bcol = persist.tile([128, BC_N], F32)
        degc = persist.tile([128, T], F32)
        S_nm = persist.tile([128, T * K], F16)
        lgs_nm = persist.tile([128, T * K], F16)
        out_fm = persist.tile([128, 12 * G], F16)  # readout maxes, col=ch*G+g
        nmax = persist.tile([128, 2], F32)
        sumx = persist.tile([128, 2], F32)
        y_sb = persist.tile([128, 2 * G], F16)
        z_sb = persist.tile([10, G], F32)

        def ones_at(p, n):
            return rows2[p : p + 1, 0:n]

        def rrow(ro, n):
            p, off = ro
            return rows2[p : p + 1, off : off + n]

        # ---------- pools (opened in LIFO close order; fnmp closes first) ----------
        hres = ex.enter_context(tc.tile_pool(name="hres", bufs=1))
        xfm_p = ex.enter_context(tc.tile_pool(name="xfm", bufs=2))
        xnm_p = ex.enter_context(tc.tile_pool(name="xnm", bufs=2))
        agg_p = ex.enter_context(tc.tile_pool(name="aggfm", bufs=2))
        mid_p = ex.enter_context(tc.tile_pool(name="midp", bufs=1))
        fnm_p = ExitStack()
        fnmpool = fnm_p.enter_context(tc.tile_pool(name="fnmp", bufs=1))

        # ---------- input DMAs (issue order == priority order) ----------
        AT = persist.tile([128, T * NPG], F16, tag="AT", name="AT")
        featnm = fnmpool.tile([128, T * IN], F16, tag="featnm")
        featT = fnmpool.tile([128, NLOC], F16, tag="featT")
        for g in range(G):
            nc.sync.dma_start(featnm[:, g * 4 * IN : (g + 1) * 4 * IN],
                              featnm_d[:, g * 4 * IN : (g + 1) * 4 * IN])
            nc.sync.dma_start(AT[:, g * 4 * NPG : (g + 1) * 4 * NPG],
                              at_d[:, g * 4 * NPG : (g + 1) * 4 * NPG])
        nc.sync.dma_start(featT[:], featT_d[:])
        W1 = wload(persist, "W1", 256, 256)
        aW1 = wload(persist, "aW1", 256, 256)
        nc.sync.dma_start(identr[:], identr_d[:])
        nc.sync.dma_start(rows2[:], rows_d[:])
        nc.sync.dma_start(bcol[:], bcol_d[:])
        nc.sync.dma_start(degc[:], degc_d[:])
        W2 = wload(persist, "W2", 512, 256)
        aW2 = wload(persist, "aW2", 512, 256)
        W3 = wload(persist, "W3", 512, 256)
        aW3 = persist.tile([128, 4 * 2048], F8, tag="aW3", name="aW3")
        aW3v = aW3[:].rearrange("p (k f) -> p k f", k=4, f=2048)
        for q in range(4):
            nc.sync.dma_start(
                aW3v[:, :, q * 512 : (q + 1) * 512],
                w_d["aW3"][:, q * 512 : (q + 1) * 512].rearrange(
                    "(k p) f -> p k f", p=128))
        pWa = wload(persist, "pWa", 512, 256)
        pW3 = wload(persist, "pW3", 2048, 256, F8)
        pW3v = pW3[:].rearrange("p (k f) -> p k f", k=16, f=256)
        qW1 = wload(persist, "qW1", 1536, 256)
        qW2 = wload(persist, "qW2", 512, 256)
        qW3 = wload(persist, "qW3", 512, 256)
        mW1 = wload(persist, "mW1", 1536, 256)
        mW2 = wload(persist, "mW2", 256, 10)

        # ---------- activation tiles ----------
        h1n = hres.tile([128, T * HID], F16, tag="h1n", name="h1n")
        h2n = hres.tile([128, T * HID], F16, tag="h2n", name="h2n")
        h3n = hres.tile([128, T * HID], F16, tag="h3n", name="h3n")
        a1f = hres.tile([128, 2 * NLOC], F16, tag="a1f", name="a1f")
        a2f = hres.tile([128, 2 * NLOC], F16, tag="a2f", name="a2f")
        a2f8 = hres.tile([128, 2 * NLOC], F8, tag="a2f8", name="a2f8")

        # ---------- PSUM evacuation, round-robin across Act/DVE ----------
        rr = [0]

        def evac(dst, src, bias=None, relu=False, w=(1, 1)):
            """dst = act(src + bias); engines weighted (Act, DVE)."""
            sel = rr[0] % (w[0] + w[1])
            rr[0] += 1
            if sel < w[0]:
                nc.scalar.activation(dst, src, AF.Relu if relu else AF.Identity,
                                     bias=bias if bias is not None else 0.0)
            else:
                if bias is None and not relu:
                    nc.vector.tensor_copy(dst, src)
                elif relu:
                    nc.vector.tensor_scalar(dst, src,
                                            bias if bias is not None else 0.0,
                                            0.0, op0=ALU.add, op1=ALU.max)
                else:
                    nc.vector.tensor_scalar(dst, src, bias, None, op0=ALU.add)

        # ---------- emit helpers ----------
        def emit_agg(x_nm, D, out_t):
            """out_t[d, n] (feature-major) = sum_s x_nm[s, d] * AT[s, n]."""
            for g in range(G):
                for ch in range(D // 128):
                    ps = ps_big()
                    for st in range(4):
                        t = g * 4 + st
                        nc.tensor.matmul(
                            ps[:],
                            lhsT=x_nm[:, t * D + ch * 128 : t * D + ch * 128 + 128],
                            rhs=AT[:, t * NPG : (t + 1) * NPG],
                            start=(st == 0), stop=(st == 3))
                    evac(out_t[:, ch * NLOC + g * NPG : ch * NLOC + (g + 1) * NPG],
                         ps[:])

        def emit_lin_fm(x_fm, a_fm, Din, Dout, Wsb, bccol, relu, out_t,
                        out2_t=None):
            nk = Din // 128
            for co in range(Dout // 128):
                for nb in range(4):
                    ps = ps_big()
                    ki = 0
                    for src in (x_fm, a_fm):
                        for ci in range(nk):
                            nc.tensor.matmul(
                                ps[:],
                                lhsT=Wsb[:, ki * Dout + co * 128 : ki * Dout + co * 128 + 128],
                                rhs=src[:, ci * NLOC + nb * 512 : ci * NLOC + (nb + 1) * 512],
                                start=(ki == 0), stop=(ki == 2 * nk - 1))
                            ki += 1
                    sl = slice(co * NLOC + nb * 512, co * NLOC + (nb + 1) * 512)
                    evac(out_t[:, sl], ps[:],
                         bias=bcol[:, bccol + co : bccol + co + 1], relu=relu)
                    if out2_t is not None:
                        evac(out2_t[:, sl], ps[:],
                             bias=bcol[:, bccol + co : bccol + co + 1], relu=relu)

        def emit_nm_T(x_fm, out_nm, dt=F16):
            # node-major via PE transposes of the (already relu'd) fm tensor;
            # 4 transposed blocks share one PSUM bank -> single 512-wide evac.
            for t2 in range(0, T, 2):
                tp = ps_big(dt)
                for i, (t, ch) in enumerate(
                        ((t2, 0), (t2, 1), (t2 + 1, 0), (t2 + 1, 1))):
                    nc.tensor.matmul(
                        tp[:, i * 128 : (i + 1) * 128],
                        lhsT=x_fm[:, ch * NLOC + t * 128 : (t + 1) * 128 + ch * NLOC],
                        rhs=identr[:], is_transpose=True,
                        start=True, stop=True, skip_group_check=True)
                evac(out_nm[:, t2 * HID : (t2 + 2) * HID], tp[:])

        def emit_out1(x_fm, ch0):
            for ci in range(2):
                for g in range(G):
                    nc.vector.tensor_reduce(
                        out_fm[:, (ch0 + ci) * G + g : (ch0 + ci) * G + g + 1],
                        x_fm[:, ci * NLOC + g * NPG : ci * NLOC + (g + 1) * NPG],
                        axis=AX.X, op=ALU.max)

        # ---------- GC stacks ----------
        aggfeat = agg_p.tile([128, NLOC], F16, tag="agg", name="aggfeat")
        emit_agg(featnm, IN, aggfeat)

        h1f = xfm_p.tile([128, 2 * NLOC], F16, tag="xfm", name="h1f")
        emit_lin_fm(featT, aggfeat, 128, 256, W1, BC_B1, True, h1f)
        emit_nm_T(h1f, h1n)
        emit_out1(h1f, 0)

        a1n = xnm_p.tile([128, T * HID], F16, tag="xnm", name="a1n")
        emit_lin_fm(featT, aggfeat, 128, 256, aW1, BC_AB1, True, a1f)
        emit_nm_T(a1f, a1n)
        fnm_p.close()

        aggh1 = agg_p.tile([128, 2 * NLOC], F16, tag="agg2", name="aggh1")
        emit_agg(h1n, HID, aggh1)

        agga1 = agg_p.tile([128, 2 * NLOC], F16, tag="agg2", name="agga1")
        emit_agg(a1n, HID, agga1)

        h2f = xfm_p.tile([128, 2 * NLOC], F16, tag="xfm", name="h2f")
        emit_lin_fm(h1f, aggh1, 256, 256, W2, BC_B2, True, h2f)
        emit_nm_T(h2f, h2n)
        emit_out1(h2f, 2)

        a2n = xnm_p.tile([128, T * HID], F16, tag="xnm", name="a2n")
        emit_lin_fm(a1f, agga1, 256, 256, aW2, BC_AB2, True, a2f, out2_t=a2f8)
        emit_nm_T(a2f, a2n)

        aggh2 = agg_p.tile([128, 2 * NLOC], F16, tag="agg2", name="aggh2")
        emit_agg(h2n, HID, aggh2)

        agga2 = agg_p.tile([128, 2 * NLOC], F8, tag="agg2", name="agga2")
        emit_agg(a2n, HID, agga2)

        # h3: fm + readout + node-major (resident, no spill)
        h3f = xfm_p.tile([128, 2 * NLOC], F16, tag="xfm", name="h3f")
        emit_lin_fm(h2f, aggh2, 256, 256, W3, BC_B3, False, h3f)
        emit_nm_T(h3f, h3n)
        emit_out1(h3f, 4)

        # ---------- a3 + logits (streamed per graph, fp8 DoubleRow) ----------
        a2f8v = a2f8[:].rearrange("p (c n) -> p c n", c=2, n=NLOC)
        agga2v = agga2[:].rearrange("p (c n) -> p c n", c=2, n=NLOC)
        for g in range(G):
            lps = lg_p.tile([64, 512], F32, tag="lg", name=_nm("lg"))
            gn = slice(g * NPG, (g + 1) * NPG)
            for cop in range(8):  # a3 = relu(cat(a2, agg_a2) @ aW3 + ab3)
                ab2 = mid_p.tile([128, 2 * 512], F8, tag="a3buf",
                                 name=_nm("a3b"), bufs=3)
                for sub in range(2):
                    co = 2 * cop + sub
                    ps3 = ps_big()
                    for pair, srcv in enumerate((a2f8v, agga2v)):
                        nc.tensor.matmul(
                            ps3[:],
                            lhsT=aW3v[:, 2 * pair : 2 * pair + 2,
                                      co * 128 : co * 128 + 128],
                            rhs=srcv[:, :, gn],
                            start=(pair == 0), stop=(pair == 1),
                            perf_mode=DR)
                    evac(ab2[:, sub * 512 : (sub + 1) * 512], ps3[:],
                         bias=bcol[:, BC_AB3 + co : BC_AB3 + co + 1], relu=True)
                ab2v = ab2[:].rearrange("p (c n) -> p c n", c=2, n=512)
                nc.tensor.matmul(
                    lps[:],
                    lhsT=pW3v[:, 2 * cop : 2 * cop + 2, g * K : g * K + K],
                    rhs=ab2v[:], start=(cop == 0), stop=False,
                    perf_mode=DR, skip_group_check=True)
            for ci in range(2):  # a1 block of pW
                nc.tensor.matmul(
                    lps[:], lhsT=pWa[:, ci * 256 + g * K : ci * 256 + g * K + K],
                    rhs=a1f[:, ci * NLOC + g * NPG : ci * NLOC + (g + 1) * NPG],
                    start=False, stop=False, skip_group_check=True)
            for ci in range(2):  # a2 block
                nc.tensor.matmul(
                    lps[:], lhsT=pWa[:, (2 + ci) * 256 + g * K : (2 + ci) * 256 + g * K + K],
                    rhs=a2f8[:, ci * NLOC + g * NPG : ci * NLOC + (g + 1) * NPG],
                    start=False, stop=False, skip_group_check=True)
            nc.tensor.matmul(lps[:],
                             lhsT=rrow(R_PB, 256)[:, g * K : (g + 1) * K],
                             rhs=ones_at(32, 512), start=False, stop=True,
                             skip_group_check=True)
            lgf = mid_p.tile([64, 512], F16, tag="lgf", name=_nm("lgf"), bufs=1)
            evac(lgf[:], lps[:])
            for j in range(4):  # transpose to node-major
                t = g * 4 + j
                tps = ps_med(128, 64, F16)
                nc.tensor.matmul(tps[:], lhsT=lgf[0:64, j * 128 : (j + 1) * 128],
                                 rhs=identr[0:64, 0:64], is_transpose=True,
                                 start=True, stop=True)
                nc.vector.tensor_copy(lgs_nm[:, t * K : (t + 1) * K], tps[:])

        # masked softmax == per-graph softmax over K columns
        for t in range(T):
            bb = t % 2
            nc.vector.tensor_reduce(nmax[:, bb : bb + 1], lgs_nm[:, t * K : (t + 1) * K],
                                    axis=AX.X, op=ALU.max, negate=True)
            nc.scalar.activation(S_nm[:, t * K : (t + 1) * K],
                                 lgs_nm[:, t * K : (t + 1) * K], AF.Exp,
                                 bias=nmax[:, bb : bb + 1],
                                 accum_out=sumx[:, bb : bb + 1])
            nc.vector.reciprocal(sumx[:, bb : bb + 1], sumx[:, bb : bb + 1])
            nc.vector.tensor_scalar(S_nm[:, t * K : (t + 1) * K],
                                    S_nm[:, t * K : (t + 1) * K],
                                    sumx[:, bb : bb + 1], None, op0=ALU.mult)

        # ---------- late pool: pooled stage ----------
        late = ex.enter_context(tc.tile_pool(name="late", bufs=1))
        Xr = [h1n, h2n, h3n]
        AS_nm = late.tile([128, T * K], F16, tag="AS", name="AS_nm")
        rs_n = late.tile([128, T], F16, tag="rsn", name="rs_n")

        # AS = A @ S: scaled-AT product un-scaled by clamped deg (exact);
        # rs_n[n] = sum_l AS[n, l] (for adj row sums)
        for t in range(T):
            g, j = t // 4, t % 4
            ps = ps_big()
            for st in range(4):
                nc.tensor.matmul(
                    ps[:, 0:K],
                    lhsT=AT[:, (g * 4 + st) * NPG + j * 128 : (g * 4 + st) * NPG + (j + 1) * 128],
                    rhs=S_nm[:, (g * 4 + st) * K : (g * 4 + st + 1) * K],
                    start=(st == 0), stop=(st == 3))
            nc.vector.tensor_scalar(AS_nm[:, t * K : (t + 1) * K], ps[:, 0:K],
                                    degc[:, t : t + 1], None, op0=ALU.mult)
            nc.vector.tensor_reduce(rs_n[:, t : t + 1],
                                    AS_nm[:, t * K : (t + 1) * K],
                                    axis=AX.X, op=ALU.add)

        # ---------- h_pool = S^T X, pair-stacked [128 = 2 graphs, .] ----------
        # gs=1 matmuls land directly on PSUM partitions 64.. (tile_position).
        hp_nm = late.tile([128, 2 * 768], F16, tag="hpn", name="hp_nm")
        hp_fm = late.tile([128, 6 * 256], F16, tag="hpf", name="hp_fm")
        for h in range(2):
            for L in range(3):
                ps = ps_big()
                for gs in range(2):
                    g = h * 2 + gs
                    for j in range(4):
                        t = g * 4 + j
                        nc.tensor.matmul(
                            ps[gs * 64 : gs * 64 + 64, 0:256],
                            lhsT=S_nm[:, t * K : (t + 1) * K],
                            rhs=Xr[L][:, t * HID : (t + 1) * HID],
                            start=(j == 0), stop=(j == 3),
                            skip_group_check=True)
                evac(hp_nm[:, h * 768 + L * 256 : h * 768 + (L + 1) * 256],
                     ps[:, 0:256])
            for ch in range(6):  # hp_fm via transposes of the pair tile
                tp = ps_med(128, 128, F16)
                nc.tensor.matmul(
                    tp[:], lhsT=hp_nm[:, h * 768 + ch * 128 : h * 768 + (ch + 1) * 128],
                    rhs=identr[:], is_transpose=True, start=True, stop=True)
                nc.vector.tensor_copy(
                    hp_fm[:, ch * 256 + h * 128 : ch * 256 + (h + 1) * 128], tp[:])

        # ---------- adjT = (AS)^T S directly (block-diag, pair-stacked) ----------
        # row sums of adj via rsum_row[1, K] = sum_n rs_n[n] S[n, k];
        # normalization applied as a column scale through a ones-outer-product.
        adjT = late.tile([128, 2 * 128], F16, tag="adjT", name="adjT")
        rrec = late.tile([1, 256], F16, tag="rrec", name="rrec")
        nc.vector.memset(adjT[:], 0.0)
        for h in range(2):
            pt = ps_big()
            pr = ps_sml(1, 256)
            for gs in range(2):
                g = h * 2 + gs
                for j in range(4):
                    t = g * 4 + j
                    nc.tensor.matmul(
                        pt[gs * 64 : gs * 64 + 64, gs * 64 : gs * 64 + 64],
                        lhsT=AS_nm[:, t * K : (t + 1) * K],
                        rhs=S_nm[:, t * K : (t + 1) * K],
                        start=(j == 0), stop=(j == 3), skip_group_check=True)
                    nc.tensor.matmul(
                        pr[0:1, gs * K : (gs + 1) * K],
                        lhsT=rs_n[:, t : t + 1],
                        rhs=S_nm[:, t * K : (t + 1) * K],
                        start=(j == 0), stop=(j == 3), skip_group_check=True)
            for gs in range(2):
                nc.vector.tensor_copy(
                    adjT[gs * 64 : gs * 64 + 64,
                         h * 128 + gs * 64 : h * 128 + gs * 64 + 64],
                    pt[gs * 64 : gs * 64 + 64, gs * 64 : gs * 64 + 64])
            nc.vector.tensor_scalar(rrec[:, h * 128 : (h + 1) * 128],
                                    pr[:, 0:128], 1e-9, None, op0=ALU.add)
            nc.vector.reciprocal(rrec[:, h * 128 : (h + 1) * 128],
                                 rrec[:, h * 128 : (h + 1) * 128])
        bcst = late.tile([128, 256], F16, tag="bcst", name="bcst")
        for h in range(2):
            pb = ps_med(128, 128)
            nc.tensor.matmul(pb[:], lhsT=ones_at(0, 128),
                             rhs=rrec[:, h * 128 : (h + 1) * 128],
                             start=True, stop=True)
            nc.scalar.copy(bcst[:, h * 128 : (h + 1) * 128], pb[:])
        nc.vector.tensor_tensor(adjT[:], in0=adjT[:], in1=bcst[:], op=ALU.mult)

        # ---------- pooled sage stack (pair-batched) ----------
        hn1_fm = late.tile([128, 6 * 256], F16, tag="hn1", name="hn1_fm")
        p1_nm = late.tile([128, 2 * 256], F16, tag="p1n", name="p1_nm")
        p1_fm = late.tile([128, 2 * 256], F16, tag="p1f", name="p1_fm")
        hn2_fm = late.tile([128, 2 * 256], F16, tag="hn2", name="hn2_fm")
        p2_nm = late.tile([128, 2 * 256], F16, tag="p2n", name="p2_nm")
        p2_fm = late.tile([128, 2 * 256], F16, tag="p2f", name="p2_fm")
        hn3_fm = late.tile([128, 2 * 256], F16, tag="hn3", name="hn3_fm")
        p3_fm = late.tile([128, 2 * 256], F16, tag="p3f", name="p3_fm")

        def pool_hn(x_nm, xw, out_t):
            # out[d, u-pair] = sum_{v-pair} x_nm[v, d] * adjT_bd[v, u]
            for h in range(2):
                for ch in range(xw // 128):
                    tp = ps_big()
                    nc.tensor.matmul(
                        tp[:, 0:128],
                        lhsT=x_nm[:, h * xw + ch * 128 : h * xw + (ch + 1) * 128],
                        rhs=adjT[:, h * 128 : (h + 1) * 128],
                        start=True, stop=True)
                    evac(out_t[:, ch * 256 + h * 128 : ch * 256 + (h + 1) * 128],
                         tp[:, 0:128])

        def pool_lin(xf, hf, Din, Wsb, bccol, rbias, relu, outf, outn):
            nch = Din // 256
            for co in range(2):
                ps = ps_big()
                ki = 0
                for src in (xf, hf):
                    for ch in range(nch):
                        nc.tensor.matmul(
                            ps[:, 0:256],
                            lhsT=Wsb[:, ki * 256 + co * 128 : ki * 256 + co * 128 + 128],
                            rhs=src[:, ch * 256 : (ch + 1) * 256],
                            start=(ki == 0), stop=(ki == 2 * nch - 1))
                        ki += 1
                evac(outf[:, co * 256 : (co + 1) * 256], ps[:, 0:256],
                     bias=bcol[:, bccol + co : bccol + co + 1], relu=relu)
            if outn is not None:
                for h in range(2):
                    ps = ps_big()
                    ki = 0
                    for src in (xf, hf):
                        for ch in range(nch):
                            nc.tensor.matmul(
                                ps[:, 0:256],
                                lhsT=src[:, ch * 256 + h * 128 : ch * 256 + (h + 1) * 128],
                                rhs=Wsb[:, ki * 256 : (ki + 1) * 256],
                                start=(ki == 0), stop=False)
                            ki += 1
                    nc.tensor.matmul(ps[:, 0:256], lhsT=ones_at(rbias[0], 128),
                                     rhs=rrow(rbias, 256),
                                     start=False, stop=True)
                    nc.vector.tensor_scalar(outn[:, h * 256 : (h + 1) * 256],
                                            ps[:, 0:256],
                                            0.0, None, op0=ALU.max)

        pool_hn(hp_nm, 768, hn1_fm)
        pool_lin(hp_fm, hn1_fm, 1536, qW1, BC_QB1, R_QB1, True, p1_fm, p1_nm)
        pool_hn(p1_nm, 256, hn2_fm)
        pool_lin(p1_fm, hn2_fm, 512, qW2, BC_QB2, R_QB2, True, p2_fm, p2_nm)
        pool_hn(p2_nm, 256, hn3_fm)
        pool_lin(p2_fm, hn3_fm, 512, qW3, BC_QB3, R_QB3, False, p3_fm, None)
        for L, pf in enumerate((p1_fm, p2_fm, p3_fm)):
            for co in range(2):
                for g in range(G):
                    nc.vector.tensor_reduce(
                        out_fm[:, (6 + L * 2 + co) * G + g : (6 + L * 2 + co) * G + g + 1],
                        pf[:, co * 256 + g * K : co * 256 + (g + 1) * K],
                        axis=AX.X, op=ALU.max)

        # ---------- final MLP ----------
        for co in range(2):
            ps = ps_sml(128, G)
            for k in range(12):
                nc.tensor.matmul(
                    ps[:], lhsT=mW1[:, k * 256 + co * 128 : k * 256 + co * 128 + 128],
                    rhs=out_fm[:, k * G : (k + 1) * G],
                    start=(k == 0), stop=(k == 11))
            nc.scalar.activation(y_sb[:, co * G : (co + 1) * G], ps[:], AF.Identity,
                                 bias=bcol[:, BC_MB1 + co : BC_MB1 + co + 1])
        zps = ps_sml(10, G)
        for ci in range(2):
            nc.tensor.matmul(zps[:], lhsT=mW2[:, ci * 10 : (ci + 1) * 10],
                             rhs=y_sb[:, ci * G : (ci + 1) * G],
                             start=(ci == 0), stop=(ci == 1))
        nc.scalar.activation(z_sb[:], zps[:], AF.Identity,
                             bias=bcol[0:10, BC_MB2 : BC_MB2 + 1])
        nc.sync.dma_start(yp_d[:], z_sb[:])

    nc.compile()
    return nc


# ---------------------------------------------------------------------------
# host side
# ---------------------------------------------------------------------------

def _pack_bcol(b):
    bc = np.zeros((128, BC_N), np.float32)
    for off, k in ((BC_B1, "b1"), (BC_B2, "b2"), (BC_B3, "b3"), (BC_AB1, "ab1"),
                   (BC_AB2, "ab2"), (BC_AB3, "ab3"), (BC_QB1, "qb1"),
                   (BC_QB2, "qb2"), (BC_QB3, "qb3"), (BC_MB1, "mb1")):
        v = np.asarray(b[k], np.float32)
        bc[:, off : off + v.size // 128] = v.reshape(-1, 128).T
    mb2 = np.asarray(b["mb2"], np.float32)
    bc[: mb2.size, BC_MB2] = mb2
    return bc


def _pack_rows(b, pb_lc):
    r = np.zeros((65, ROWS_W), np.float32)
    for p in (0, 32, 64):
        r[p, 0:512] = 1.0
    for (p, off), k in ((R_QB1, "qb1"), (R_QB2, "qb2"), (R_QB3, "qb3")):
        r[p, off : off + 256] = b[k]
    p, off = R_PB
    r[p, off : off + 256] = pb_lc
    return r.astype(np.float16)


def _at_dense(edge_src, edge_dst, core):
    """Dense scaled A^T tiles [128, T*NPG] fp16 plus clamped-deg cols."""
    lo, hi = core * NLOC, (core + 1) * NLOC
    m = (edge_dst >= lo) & (edge_dst < hi)
    src = edge_src[m].astype(np.int64)
    dst = edge_dst[m].astype(np.int64)
    gg = dst // NPG
    if not np.array_equal(src // NPG, gg):
        raise ValueError("cross-graph edges break graph-parallel sharding")
    gl = gg - core * G
    sl = src - gg * NPG
    dl = dst - gg * NPG
    t = gl * 4 + sl // 128
    p = sl % 128
    flat = (p * T + t) * NPG + dl
    cnt = np.bincount(flat, minlength=128 * T * NPG).astype(np.float64)
    at = cnt.reshape(128, T * NPG)
    # deg per local node (node-major: node = tt*128 + pp)
    nl = gl * NPG + dl
    deg = np.bincount(nl, minlength=NLOC).astype(np.float64)
    degc = np.maximum(deg, 1.0)
    # scale each AT column (dst d of graph g == local node g*NPG+d)
    colnode = (np.arange(T * NPG) // (4 * NPG)) * NPG + np.arange(T * NPG) % NPG
    at = at / degc[colnode][None, :]
    degc_nm = degc.reshape(T, 128).T.astype(np.float32)
    return at.astype(np.float16), np.ascontiguousarray(degc_nm)


_CACHE = {}
TRACE = False


def prepare_in_maps(inputs):
    import ml_dtypes
    f16 = lambda x: np.ascontiguousarray(np.asarray(x, np.float32).astype(np.float16))
    f8 = lambda x: np.ascontiguousarray(
        np.asarray(x, np.float32).astype(ml_dtypes.float8_e4m3))
    feat = np.asarray(inputs["feat"], np.float32)
    edge_src = np.asarray(inputs["edge_src"])
    edge_dst = np.asarray(inputs["edge_dst"])
    W = {k: f16(inputs[k]) for k in
         ("W1", "W2", "W3", "aW1", "aW2", "pW", "qW1", "qW2", "qW3",
          "mW1", "mW2")}
    b = {k: np.asarray(inputs[k], np.float32) for k in
         ("b1", "b2", "b3", "ab1", "ab2", "ab3", "pb", "qb1", "qb2", "qb3",
          "mb1", "mb2")}
    aW3_8 = f8(inputs["aW3"])
    identr = np.eye(128, dtype=np.float16)
    bcol = _pack_bcol(b)

    in_maps = []
    for c in range(NCORES):
        fs = feat[c * NLOC : (c + 1) * NLOC]
        feat_nm = np.ascontiguousarray(
            fs.reshape(T, 128, IN).transpose(1, 0, 2).reshape(128, T * IN))
        featT = np.ascontiguousarray(fs.T)
        at, degc = _at_dense(edge_src, edge_dst, c)
        pW_lc = np.ascontiguousarray(W["pW"][:, c * G * K : (c + 1) * G * K])
        pb_lc = np.ascontiguousarray(b["pb"][c * G * K : (c + 1) * G * K])
        in_maps.append({
            "featT": f16(featT), "feat_nm": f16(feat_nm),
            "at_dense": at, "degc": degc,
            "bcol": bcol, "rows2": _pack_rows(b, pb_lc),
            "identr": identr,
            "W1": W["W1"], "W2": W["W2"], "W3": W["W3"],
            "aW1": W["aW1"], "aW2": W["aW2"], "aW3": aW3_8,
            "pWa": np.ascontiguousarray(pW_lc[:512]),
            "pW3": f8(pW_lc[512:]),
            "qW1": W["qW1"], "qW2": W["qW2"], "qW3": W["qW3"],
            "mW1": W["mW1"], "mW2": W["mW2"],
        })
    return in_maps


def kernel(**inputs):
    if "nc" not in _CACHE:
        _CACHE["nc"] = build_module()
    nc = _CACHE["nc"]
    in_maps = prepare_in_maps(inputs)
    res = run_bass_kernel_spmd(nc, in_maps, core_ids=list(range(NCORES)),
                               trace=TRACE)
    _CACHE["last_res"] = res
    out = np.zeros((B, 10), np.float32)
    for c in range(NCORES):
        out[c * G : (c + 1) * G, :] = np.asarray(res.results[c]["yp"]).T
    return out
